# revision 2
# baseline (speedup 1.0000x reference)
"""DEQ transformer block with Anderson acceleration on 8 Trainium2 NeuronCores.

v2: fp16 matmul pipeline.  Each of the 4 sequences (B=4) is split across a
pair of cores (512 tokens each).  K/V halves are exchanged within each pair
via AllGather (fp16) every DEQ iteration.  Attention/in-proj weights are
SBUF-resident fp16 (LayerNorm gamma/beta folded in host-side); MLP weights
stream per iteration.  Softmax denominators ride along the AV matmul via a
ones column appended to V.  All accumulation in fp32 PSUM; LN stats,
softmax normalization and the Anderson solve run in fp32.
"""

import numpy as np

P = 128
TL = 512          # tokens per core (half a sequence)
C = 768
CCN = 6           # C / 128
NH = 12
DH = 64
HPN = 6           # head pairs
NHID = 3072
HCN = 24          # NHID / 128
KCN = 8           # full-seq key chunks (1024 / 128)
MH = 5            # Anderson history window
LN_EPS = 1e-5
NCORES = 8
GROUPS = [[0, 1], [2, 3], [4, 5], [6, 7]]

_CACHE = {}


def _build(num_iters):
    from contextlib import ExitStack
    import concourse.bass as bass  # noqa
    import concourse.mybir as mybir
    import concourse.tile as tile
    from concourse import bacc
    from concourse.masks import make_identity

    FP = mybir.dt.float32
    FPR = mybir.dt.float32r
    H = mybir.dt.float16
    AF = mybir.ActivationFunctionType
    OP = mybir.AluOpType

    nc = bacc.Bacc()

    # ---------------- DRAM I/O ----------------
    u_d = nc.dram_tensor("u_fm", [C, TL], H, kind="ExternalInput")
    qkw_d = nc.dram_tensor("qkw_pack", [12, P, CCN, P], H, kind="ExternalInput")
    vw_d = nc.dram_tensor("vw_pack", [P, CCN, C], H, kind="ExternalInput")
    wo_d = nc.dram_tensor("wo_pack", [CCN, P, CCN, P], H, kind="ExternalInput")
    w1_d = nc.dram_tensor("w1_pack", [HCN, P, CCN, P], H, kind="ExternalInput")
    w2_d = nc.dram_tensor("w2_pack", [CCN, P, HCN, P], H, kind="ExternalInput")
    vb_d = nc.dram_tensor("vbias_bc", [P, C], FP, kind="ExternalInput")
    bqk_d = nc.dram_tensor("bqk_cols", [P, 12], FP, kind="ExternalInput")
    bo_d = nc.dram_tensor("bo_cols", [P, CCN], FP, kind="ExternalInput")
    b1_d = nc.dram_tensor("b1_cols", [P, HCN], FP, kind="ExternalInput")
    b2_d = nc.dram_tensor("b2_cols", [P, CCN], FP, kind="ExternalInput")
    emb_d = nc.dram_tensor("emb_cols", [P, num_iters * CCN], FP, kind="ExternalInput")
    eye_d = nc.dram_tensor("eye28_c", [P, 14, 14], H, kind="ExternalInput")
    o1h_d = nc.dram_tensor("ones1h_c", [1, P], H, kind="ExternalInput")
    von_d = nc.dram_tensor("vones_c", [P, KCN * NH], H, kind="ExternalInput")
    zo_d = nc.dram_tensor("z_out", [C, TL], FP, kind="ExternalOutput")

    # internal DRAM
    kcc = nc.dram_tensor("k_cc", [C, TL], H)
    vcc = nc.dram_tensor("v_cc", [TL, C], H)
    kall = nc.dram_tensor("k_all", [2, C, TL], H)
    vall = nc.dram_tensor("v_all", [2, TL, C], H)
    fh = nc.dram_tensor("f_hist", [MH, C, TL], FP)

    with tile.TileContext(nc) as tc:
        ctx = ExitStack()
        pool = ctx.enter_context(tc.tile_pool(name="pers", bufs=1))
        itp = ctx.enter_context(tc.tile_pool(name="itp", bufs=1))
        vec = ctx.enter_context(tc.tile_pool(name="vec", bufs=3))
        attp = ctx.enter_context(tc.tile_pool(name="attp", bufs=4))
        w1p = ctx.enter_context(tc.tile_pool(name="w1p", bufs=4))
        w2p = ctx.enter_context(tc.tile_pool(name="w2p", bufs=2))
        wkp = ctx.enter_context(tc.tile_pool(name="wkp", bufs=4))
        fpool = ctx.enter_context(tc.tile_pool(name="fpool", bufs=4))
        prodp = ctx.enter_context(tc.tile_pool(name="prodp", bufs=2))
        rowp = ctx.enter_context(tc.tile_pool(name="rowp", bufs=2))
        vrow = ctx.enter_context(tc.tile_pool(name="vrow", bufs=4))
        pA = ctx.enter_context(tc.tile_pool(name="pA", bufs=2, space="PSUM"))
        pB = ctx.enter_context(tc.tile_pool(name="pB", bufs=2, space="PSUM"))
        pC = ctx.enter_context(tc.tile_pool(name="pC", bufs=2, space="PSUM"))

        # ------------- persistent tiles -------------
        vw_sb = pool.tile([P, CCN, C], H, name="vw_sb")
        z_sb = pool.tile([P, CCN, TL], FP, name="z_sb")
        k_sb = pool.tile([P, CCN, 2 * TL], H, name="k_sb")
        v_sb = pool.tile([P, KCN, NH, DH + 1], H, name="v_sb")
        qo_sb = pool.tile([P, CCN, TL], H, name="qo_sb")   # Q, then attn out
        xn_sb = pool.tile([P, CCN, TL], H, name="xn_sb")   # x1, then x2
        attnres = pool.tile([P, CCN, TL], FP, name="attnres")
        g_sb = pool.tile([P, HCN, TL], H, name="g_sb")
        coefbc = pool.tile([P, MH, TL], H, name="coefbc")
        bqk_sb = pool.tile([P, 12], FP, name="bqk_sb")
        bo_sb = pool.tile([P, CCN], FP, name="bo_sb")
        b1_sb = pool.tile([P, HCN], FP, name="b1_sb")
        b2_sb = pool.tile([P, CCN], FP, name="b2_sb")
        emb_sb = pool.tile([P, num_iters * CCN], FP, name="emb_sb")
        vbbc = pool.tile([P, C], FP, name="vbbc")
        u_sb = pool.tile([P, CCN, TL], H, name="u_sb")
        ident = pool.tile([P, P], FP, name="ident")
        ones1h = pool.tile([1, P], H, name="ones1h")
        ones1r = pool.tile([1, P], FPR, name="ones1r")
        ones2 = pool.tile([P, 2], FPR, name="ones2")
        eye28 = pool.tile([P, 14, 14], H, name="eye28")
        srows = pool.tile([28, TL], FP, name="srows")
        dots_tm = pool.tile([P, 4, 28], FP, name="dots_tm")
        work = pool.tile([P, 4, 28], FP, name="work")
        coef_tm = pool.tile([P, 4, MH], FP, name="coef_tm")
        crows = pool.tile([MH, TL], FPR, name="crows")

        # loads of persistent weights / consts
        nc.sync.dma_start(vw_sb[:], vw_d[:])
        nc.sync.dma_start(bqk_sb[:], bqk_d[:])
        nc.sync.dma_start(bo_sb[:], bo_d[:])
        nc.sync.dma_start(b1_sb[:], b1_d[:])
        nc.sync.dma_start(b2_sb[:], b2_d[:])
        nc.sync.dma_start(emb_sb[:], emb_d[:])
        nc.sync.dma_start(vbbc[:], vb_d[:])
        for ucc in range(CCN):
            nc.sync.dma_start(u_sb[:, ucc], u_d[ucc * P:(ucc + 1) * P, :])
        make_identity(nc, ident[:])
        nc.sync.dma_start(ones1h[:], o1h_d[:])
        onesf = vec.tile([P, P], FP, name="v")
        nc.vector.memset(onesf[:], 1.0)
        nc.scalar.copy(ones1r[:], onesf[0:1, :])
        nc.scalar.copy(ones2[:], onesf[:, 0:2])
        nc.sync.dma_start(eye28[:], eye_d[:])
        # ones column for the AV denominator ride-along
        nc.sync.dma_start(v_sb[:, :, :, DH:DH + 1], von_d[:])

        def ecol(it, cc):
            return emb_sb[:, it * CCN + cc:it * CCN + cc + 1]

        TT = nc.vector.tensor_tensor
        TS = nc.vector.tensor_scalar

        def layernorm(src, dst):
            # src [P, CCN, TL] FPR; dst [P, CCN, TL] H
            # gamma/beta are folded into the downstream projection weights.
            pmu = pA.tile([2, TL], FP, name="pg")
            pmsq = pA.tile([2, TL], FP, name="pg")
            for cc in range(CCN):
                sq = vec.tile([P, TL], FPR, name="v")
                nc.scalar.activation(sq[:], src[:, cc].bitcast(FP), AF.Square)
                nc.tensor.matmul(pmu[:], ones2[:], src[:, cc],
                                 start=(cc == 0), stop=(cc == CCN - 1))
                nc.tensor.matmul(pmsq[:], ones2[:], sq[:],
                                 start=(cc == 0), stop=(cc == CCN - 1))
            mean_r = vrow.tile([1, TL], FPR, name="vr")
            msq_r = vrow.tile([1, TL], FP, name="vr")
            var_r = vrow.tile([1, TL], FP, name="vr")
            nc.scalar.activation(mean_r[:], pmu[0:1, :], AF.Identity,
                                 scale=1.0 / C)
            nc.scalar.activation(msq_r[:], pmsq[0:1, :], AF.Identity,
                                 scale=1.0 / C)
            TT(out=var_r[:], in0=mean_r[:].bitcast(FP),
               in1=mean_r[:].bitcast(FP), op=OP.mult)
            TT(out=var_r[:], in0=msq_r[:], in1=var_r[:], op=OP.subtract)
            TS(out=var_r[:], in0=var_r[:], scalar1=LN_EPS, scalar2=None,
               op0=OP.add)
            sd_r = vrow.tile([1, TL], FP, name="vr")
            nc.scalar.activation(sd_r[:], var_r[:], AF.Sqrt)
            rstd_f = vrow.tile([1, TL], FP, name="vr")
            nc.vector.reciprocal_approx_fast(rstd_f[:], sd_r[:])
            rstd_r = vrow.tile([1, TL], FPR, name="vr")
            nc.scalar.copy(rstd_r[:], rstd_f[:])
            pmean = pA.tile([P, TL], FP, name="pg")
            prstd = pA.tile([P, TL], FP, name="pg")
            nc.tensor.matmul(pmean[:], ones1r[:], mean_r[:],
                             start=True, stop=True)
            nc.tensor.matmul(prstd[:], ones1r[:], rstd_r[:],
                             start=True, stop=True)
            pmean_sb = vec.tile([P, TL], FP, name="v")
            prstd_sb = vec.tile([P, TL], FP, name="v")
            nc.scalar.copy(pmean_sb[:], pmean[:])
            nc.scalar.copy(prstd_sb[:], prstd[:])
            for cc in range(CCN):
                t1 = vec.tile([P, TL], FP, name="v")
                TT(out=t1[:], in0=src[:, cc].bitcast(FP), in1=pmean_sb[:],
                   op=OP.subtract)
                with nc.allow_low_precision(reason="fp16 ln out"):
                    TT(out=dst[:, cc], in0=t1[:], in1=prstd_sb[:], op=OP.mult)

        hist = []  # slot ids of stored residuals, oldest..newest

        for it in range(num_iters):
            s_new = it % MH
            prev = hist[-4:]
            Kn = len(prev)

            # ---- A1: z_ctx = z + u + 0.1*emb_it ; x1 = LN1(z_ctx) ----
            zctx = itp.tile([P, CCN, TL], FPR, name="zctx")
            for cc in range(CCN):
                if it == 0:
                    TS(out=zctx[:, cc], in0=u_sb[:, cc], scalar1=ecol(it, cc),
                       scalar2=None, op0=OP.add)
                else:
                    t0 = vec.tile([P, TL], FP, name="v")
                    TS(out=t0[:], in0=z_sb[:, cc], scalar1=ecol(it, cc),
                       scalar2=None, op0=OP.add)
                    TT(out=zctx[:, cc], in0=t0[:], in1=u_sb[:, cc], op=OP.add)

            layernorm(zctx, xn_sb)

            # ---- A2: V projection (token-major), stage + AllGather ----
            for tch in range(4):
                pva = pA.tile([P, TL], FP, name="pg")
                pvb = pA.tile([P, TL], FP, name="pg")
                ts = slice(tch * P, (tch + 1) * P)
                for cc in range(CCN):
                    nc.tensor.matmul(pva[:], xn_sb[:, cc, ts], vw_sb[:, cc, 0:512],
                                     start=(cc == 0), stop=(cc == CCN - 1))
                    nc.tensor.matmul(pvb[:, 0:256], xn_sb[:, cc, ts],
                                     vw_sb[:, cc, 512:768],
                                     start=(cc == 0), stop=(cc == CCN - 1))
                with nc.allow_low_precision(reason="fp16 v"):
                    TT(out=v_sb[:, tch, 0:8, 0:DH], in0=pva[:],
                       in1=vbbc[:, 0:512], op=OP.add)
                    TT(out=v_sb[:, tch, 8:12, 0:DH], in0=pvb[:, 0:256],
                       in1=vbbc[:, 512:768], op=OP.add)
                nc.sync.dma_start(vcc[ts, :], v_sb[:, tch, :, 0:DH])

            nc.gpsimd.collective_compute(
                "AllGather", OP.bypass, replica_groups=GROUPS,
                ins=[vcc[:]], outs=[vall[:]])

            # ---- A2': K projection (local tokens), stage + AllGather ----
            for oc in range(CCN):
                wkt = wkp.tile([P, CCN, P], H, name="wkt")
                nc.sync.dma_start(wkt[:], qkw_d[6 + oc])
                pk = pA.tile([P, TL], FP, name="pg")
                for cc in range(CCN):
                    nc.tensor.matmul(pk[:], wkt[:, cc], xn_sb[:, cc],
                                     start=(cc == 0), stop=(cc == CCN - 1))
                with nc.allow_low_precision(reason="fp16 k"):
                    nc.scalar.activation(k_sb[:, oc, 0:TL], pk[:], AF.Identity,
                                         bias=bqk_sb[:, 6 + oc:7 + oc])
                nc.sync.dma_start(kcc[oc * P:(oc + 1) * P, :], k_sb[:, oc, 0:TL])

            nc.gpsimd.collective_compute(
                "AllGather", OP.bypass, replica_groups=GROUPS,
                ins=[kcc[:]], outs=[kall[:]])

            # ---- A3: Q projection (overlaps the K/V collectives) ----
            for oc in range(CCN):
                wkt = wkp.tile([P, CCN, P], H, name="wkt")
                nc.sync.dma_start(wkt[:], qkw_d[oc])
                pq = pA.tile([P, TL], FP, name="pg")
                for cc in range(CCN):
                    nc.tensor.matmul(pq[:], wkt[:, cc], xn_sb[:, cc],
                                     start=(cc == 0), stop=(cc == CCN - 1))
                with nc.allow_low_precision(reason="fp16 q"):
                    nc.scalar.activation(qo_sb[:, oc], pq[:], AF.Identity,
                                         bias=bqk_sb[:, oc:oc + 1])

            # K/V gathered readback
            for r in range(2):
                for cc in range(CCN):
                    nc.sync.dma_start(k_sb[:, cc, r * TL:(r + 1) * TL],
                                      kall[r, cc * P:(cc + 1) * P, :])
            for r in range(2):
                for tch in range(4):
                    nc.sync.dma_start(v_sb[:, 4 * r + tch, :, 0:DH],
                                      vall[r, tch * P:(tch + 1) * P, :])

            # ---- B: attention (denominator rides the AV matmul) ----
            for hp in range(HPN):
                ha, hb = 2 * hp, 2 * hp + 1
                pava = pC.tile([P, TL], FP, name="pv")
                pavb = pC.tile([P, TL], FP, name="pv")
                for kc in range(KCN):
                    ks = slice(kc * P, (kc + 1) * P)
                    s2 = pB.tile([P, 2, TL], FP, name="sc2")
                    nc.tensor.matmul(s2[:, 0], k_sb[0:64, hp, ks], qo_sb[0:64, hp],
                                     start=True, stop=True)
                    nc.tensor.matmul(s2[:, 1], k_sb[64:128, hp, ks],
                                     qo_sb[64:128, hp], start=True, stop=True)
                    att2 = attp.tile([P, 2, TL], H, name="att")
                    with nc.allow_low_precision(reason="fp16 att"):
                        nc.scalar.activation(att2[:], s2[:], AF.Exp, scale=0.125)
                    nc.tensor.matmul(pava[0:DH + 1, :], v_sb[:, kc, ha, :],
                                     att2[:, 0], start=(kc == 0), stop=(kc == KCN - 1))
                    nc.tensor.matmul(pavb[0:DH + 1, :], v_sb[:, kc, hb, :],
                                     att2[:, 1], start=(kc == 0), stop=(kc == KCN - 1))
                dh2 = rowp.tile([1, 2, TL], H, name="dh2")
                with nc.allow_low_precision(reason="fp16 den"):
                    nc.scalar.copy(dh2[:, 0], pava[DH:DH + 1, :])
                    nc.scalar.copy(dh2[:, 1], pavb[DH:DH + 1, :])
                avs = attp.tile([64, 2, TL], FP, name="avs", bufs=2)
                nc.scalar.copy(avs[:, 0], pava[0:64, :])
                nc.scalar.copy(avs[:, 1], pavb[0:64, :])
                pbc2 = pB.tile([P, 2, TL], FP, name="sc2")
                nc.tensor.matmul(pbc2[0:64, 0], ones1h[:, 0:64], dh2[:, 0],
                                 start=True, stop=True)
                nc.tensor.matmul(pbc2[0:64, 1], ones1h[:, 0:64], dh2[:, 1],
                                 start=True, stop=True)
                rbc2 = attp.tile([64, 2, TL], FP, name="rbc", bufs=2)
                nc.vector.reciprocal_approx_fast(rbc2[:], pbc2[0:64, :])
                with nc.allow_low_precision(reason="fp16 attn out"):
                    TT(out=qo_sb[0:64, hp], in0=avs[:, 0], in1=rbc2[:, 0],
                       op=OP.mult)
                    TT(out=qo_sb[64:128, hp], in0=avs[:, 1],
                       in1=rbc2[:, 1], op=OP.mult)

            # ---- C: output projection -> attnres (f32) ----
            for oc in range(CCN):
                wkt = wkp.tile([P, CCN, P], H, name="wkt")
                nc.sync.dma_start(wkt[:], wo_d[oc])
                pp = pA.tile([P, TL], FP, name="pg")
                for ci in range(CCN):
                    nc.tensor.matmul(pp[:], wkt[:, ci], qo_sb[:, ci],
                                     start=(ci == 0), stop=(ci == CCN - 1))
                nc.scalar.activation(attnres[:, oc], pp[:], AF.Identity,
                                     bias=bo_sb[:, oc:oc + 1])

            # ---- D: z_attn = z + attnres ; x2 = LN2(z_attn) ----
            za = itp.tile([P, CCN, TL], FPR, name="zctx")
            for cc in range(CCN):
                if it == 0:
                    nc.vector.tensor_copy(za[:, cc], attnres[:, cc])
                else:
                    TT(out=za[:, cc], in0=z_sb[:, cc], in1=attnres[:, cc],
                       op=OP.add)
            layernorm(za, xn_sb)

            # ---- E: MLP; res = attnres + mlp ----
            for hi in range(HCN):
                w1t = w1p.tile([P, CCN, P], H, name="w1t")
                nc.sync.dma_start(w1t[:], w1_d[hi])
                ph = pA.tile([P, TL], FP, name="pg")
                for cc in range(CCN):
                    nc.tensor.matmul(ph[:], w1t[:, cc], xn_sb[:, cc],
                                     start=(cc == 0), stop=(cc == CCN - 1))
                with nc.allow_low_precision(reason="fp16 gelu"):
                    nc.scalar.activation(g_sb[:, hi], ph[:], AF.Gelu,
                                         bias=b1_sb[:, hi:hi + 1])
            for oc in range(CCN):
                po = pC.tile([P, TL], FP, name="pv")
                for h2 in range(2):
                    w2t = w2p.tile([P, 12, P], H, name="w2t")
                    nc.sync.dma_start(w2t[:], w2_d[oc, :, h2 * 12:(h2 + 1) * 12])
                    for hj in range(12):
                        hi = h2 * 12 + hj
                        nc.tensor.matmul(po[:], w2t[:, hj], g_sb[:, hi],
                                         start=(hi == 0), stop=(hi == HCN - 1))
                t2 = vec.tile([P, TL], FP, name="v")
                TS(out=t2[:], in0=po[:], scalar1=b2_sb[:, oc:oc + 1],
                   scalar2=None, op0=OP.add)
                TT(out=attnres[:, oc], in0=attnres[:, oc], in1=t2[:], op=OP.add)

            # store res as newest history entry
            for cc in range(CCN):
                nc.sync.dma_start(fh[s_new, cc * P:(cc + 1) * P, :], attnres[:, cc])

            # ---- F: Anderson update ----
            if Kn == 0:
                for cc in range(CCN):
                    nc.vector.tensor_copy(z_sb[:, cc], attnres[:, cc])
            else:
                pairs = [(i, j) for i in range(Kn) for j in range(i, Kn)]
                pairs += [(k, Kn) for k in range(Kn)]
                NP = len(pairs)

                def dfslot(k, cc, write=False):
                    if k == 0:
                        return k_sb[:, cc, 0:TL]
                    if k == 1:
                        return k_sb[:, cc, TL:2 * TL]
                    if k == 2:
                        return qo_sb[:, cc]
                    return za[:, cc] if write else za[:, cc].bitcast(FP)

                pd = pA.tile([28, TL], FP, name="pg")
                for cc in range(CCN):
                    dfs = []
                    for k in range(Kn):
                        ft = fpool.tile([P, TL], FP, name="ft")
                        nc.sync.dma_start(
                            ft[:], fh[prev[k], cc * P:(cc + 1) * P, :])
                        with nc.allow_low_precision(reason="fp16 dF"):
                            TT(out=dfslot(k, cc, write=True), in0=ft[:],
                               in1=attnres[:, cc], op=OP.subtract)
                        dfs.append(dfslot(k, cc))
                    for idx, (a, b) in enumerate(pairs):
                        prod = prodp.tile([P, TL], H, name="prod")
                        with nc.allow_low_precision(reason="fp16 dots"):
                            if a == b:
                                nc.scalar.activation(prod[:], dfs[a],
                                                     AF.Square)
                            else:
                                ina = dfs[a]
                                inb = (dfs[b] if b < Kn
                                       else attnres[:, cc])
                                TT(out=prod[:], in0=ina, in1=inb, op=OP.mult)
                        nc.tensor.matmul(pd[0:NP, :], eye28[:, idx, 0:NP],
                                         prod[:],
                                         start=(cc == 0 and idx == 0),
                                         stop=(cc == CCN - 1 and idx == NP - 1),
                                         skip_group_check=True)
                nc.scalar.copy(srows[0:NP, :], pd[0:NP, :])

                # transpose dots to token-major
                for tch in range(4):
                    ptr = pA.tile([P, TL], FP, name="pg")
                    nc.tensor.transpose(ptr[:, 0:NP],
                                        srows[0:NP, tch * P:(tch + 1) * P],
                                        ident[0:NP, 0:NP])
                    nc.scalar.copy(dots_tm[:, tch, 0:NP], ptr[:, 0:NP])

                def pidx_of(a, b):
                    return pairs.index((min(a, b), max(a, b)))

                for a in range(Kn):
                    for b in range(Kn):
                        nc.vector.tensor_copy(work[:, :, a * 4 + b],
                                              dots_tm[:, :, pidx_of(a, b)])
                    TS(out=work[:, :, a * 4 + a], in0=work[:, :, a * 4 + a],
                       scalar1=1e-6, scalar2=None, op0=OP.add)
                    nc.vector.tensor_copy(work[:, :, 16 + a],
                                          dots_tm[:, :, pidx_of(a, Kn)])

                def As(a, b):
                    return work[:, :, a * 4 + b]

                def Bs(k):
                    return work[:, :, 16 + k]

                def Al(k):
                    return work[:, :, 20 + k]

                rin = work[:, :, 24]
                tmp = work[:, :, 25]
                fco = work[:, :, 26]
                for i in range(Kn):
                    nc.vector.reciprocal(rin, As(i, i))
                    for j in range(i + 1, Kn):
                        TT(out=fco, in0=As(j, i), in1=rin, op=OP.mult)
                        for m in range(i, Kn):
                            TT(out=tmp, in0=fco, in1=As(i, m), op=OP.mult)
                            TT(out=As(j, m), in0=As(j, m), in1=tmp,
                               op=OP.subtract)
                        TT(out=tmp, in0=fco, in1=Bs(i), op=OP.mult)
                        TT(out=Bs(j), in0=Bs(j), in1=tmp, op=OP.subtract)
                for i in range(Kn - 1, -1, -1):
                    nc.vector.tensor_copy(tmp, Bs(i))
                    for j in range(i + 1, Kn):
                        TT(out=fco, in0=As(i, j), in1=Al(j), op=OP.mult)
                        TT(out=tmp, in0=tmp, in1=fco, op=OP.subtract)
                    nc.vector.reciprocal(rin, As(i, i))
                    TT(out=Al(i), in0=tmp, in1=rin, op=OP.mult)

                # coeffs: ck = -alpha_k  (c0 = 1 + sum(alpha) folds away:
                # z_new = z + res + sum_k ck*dF_k)
                TS(out=coef_tm[:, :, 0:Kn], in0=work[:, :, 20:20 + Kn],
                   scalar1=-1.0, scalar2=None, op0=OP.mult)

                for tch in range(4):
                    ptr = pA.tile([P, TL], FP, name="pg")
                    nc.tensor.transpose(ptr[0:Kn, 0:P],
                                        coef_tm[:, tch, 0:Kn], ident[:])
                    nc.scalar.copy(crows[0:Kn, tch * P:(tch + 1) * P],
                                   ptr[0:Kn, 0:P])

                for k in range(Kn):
                    cst = rowp.tile([1, TL], FPR, name="cst")
                    nc.sync.dma_start(cst[:], crows[k:k + 1, :])
                    pb = pA.tile([P, TL], FP, name="pg")
                    nc.tensor.matmul(pb[:], ones1r[:], cst[:],
                                     start=True, stop=True)
                    with nc.allow_low_precision(reason="fp16 coef"):
                        nc.scalar.copy(coefbc[:, k, :], pb[:])

                # z += res + sum_k ck*dF_k  (dF cached in dead attn tiles)
                for cc in range(CCN):
                    TT(out=z_sb[:, cc], in0=z_sb[:, cc], in1=attnres[:, cc],
                       op=OP.add)
                    t0 = vec.tile([P, TL], FP, name="v")
                    for k in range(Kn):
                        TT(out=t0[:], in0=dfslot(k, cc),
                           in1=coefbc[:, k, :], op=OP.mult)
                        TT(out=z_sb[:, cc], in0=z_sb[:, cc], in1=t0[:],
                           op=OP.add)

            hist.append(s_new)
            if len(hist) > MH:
                hist.pop(0)

        for cc in range(CCN):
            nc.sync.dma_start(zo_d[cc * P:(cc + 1) * P, :], z_sb[:, cc])

        ctx.close()

    nc.finalize()
    return nc


def _host_pack(inputs, num_iters):
    f32 = np.float32
    f16 = np.float16
    ipw = np.ascontiguousarray(inputs["in_proj_w"], f32)
    ipb = np.ascontiguousarray(inputs["in_proj_b"], f32)
    opw = np.ascontiguousarray(inputs["out_proj_w"], f32)
    opb = np.ascontiguousarray(inputs["out_proj_b"], f32)
    w1 = np.ascontiguousarray(inputs["mlp_w1"], f32)
    b1 = np.ascontiguousarray(inputs["mlp_b1"], f32)
    w2 = np.ascontiguousarray(inputs["mlp_w2"], f32)
    b2 = np.ascontiguousarray(inputs["mlp_b2"], f32)
    emb = np.ascontiguousarray(inputs["iter_emb"], f32)
    ln1_w = np.asarray(inputs["ln1_w"], f32)
    ln1_b = np.asarray(inputs["ln1_b"], f32)
    ln2_w = np.asarray(inputs["ln2_w"], f32)
    ln2_b = np.asarray(inputs["ln2_b"], f32)

    # fold LN1 gamma/beta into in_proj, LN2 into mlp_w1
    ipw_e = ipw * ln1_w[None, :]
    ipb_e = ipb + ipw @ ln1_b
    w1_e = w1 * ln2_w[None, :]
    b1_e = b1 + w1 @ ln2_b

    qkw_pack = np.ascontiguousarray(
        ipw_e[:1536].reshape(12, P, CCN, P).transpose(0, 3, 2, 1)).astype(f16)
    vw_pack = np.ascontiguousarray(
        ipw_e[1536:].T.reshape(CCN, P, C).transpose(1, 0, 2)).astype(f16)
    wo_pack = np.ascontiguousarray(
        opw.reshape(CCN, P, CCN, P).transpose(0, 3, 2, 1)).astype(f16)
    w1_pack = np.ascontiguousarray(
        w1_e.reshape(HCN, P, CCN, P).transpose(0, 3, 2, 1)).astype(f16)
    w2_pack = np.ascontiguousarray(
        w2.reshape(CCN, P, HCN, P).transpose(0, 3, 2, 1)).astype(f16)
    vbias_bc = np.ascontiguousarray(
        np.broadcast_to(ipb_e[1536:].reshape(1, C), (P, C))).astype(f32)
    bqk_cols = np.ascontiguousarray(ipb_e[:1536].reshape(12, P).T)
    bo_cols = np.ascontiguousarray(opb.reshape(CCN, P).T)
    b1_cols = np.ascontiguousarray(b1_e.reshape(HCN, P).T)
    b2_cols = np.ascontiguousarray(b2.reshape(CCN, P).T)
    rows = [min(i, emb.shape[0] - 1) for i in range(num_iters)]
    emb_cols = np.ascontiguousarray(
        (0.1 * emb[rows]).reshape(num_iters, CCN, P).transpose(2, 0, 1)
        .reshape(P, num_iters * CCN))
    eye28_c = np.broadcast_to(np.eye(14, dtype=f16), (P, 14, 14)).copy()
    ones1h_c = np.ones((1, P), f16)
    vones_c = np.ones((P, KCN * NH), f16)
    shared = dict(
        qkw_pack=qkw_pack, vw_pack=vw_pack, wo_pack=wo_pack, w1_pack=w1_pack,
        w2_pack=w2_pack, vbias_bc=vbias_bc, bqk_cols=bqk_cols,
        bo_cols=bo_cols, b1_cols=b1_cols, b2_cols=b2_cols,
        emb_cols=emb_cols, eye28_c=eye28_c, ones1h_c=ones1h_c,
        vones_c=vones_c)
    u = np.ascontiguousarray(inputs["u"], f32)
    in_maps = []
    for core in range(NCORES):
        b, h = core // 2, core % 2
        m = dict(shared)
        m["u_fm"] = np.ascontiguousarray(u[b, h * TL:(h + 1) * TL, :].T).astype(f16)
        in_maps.append(m)
    return in_maps


def run_device(inputs, num_iters=None, trace=False):
    from concourse.bass_utils import run_bass_kernel_spmd
    ni = int(inputs.get("num_iters", 6)) if num_iters is None else num_iters
    if ni not in _CACHE:
        _CACHE[ni] = _build(ni)
    nc = _CACHE[ni]
    in_maps = _host_pack(inputs, ni)
    r = run_bass_kernel_spmd(nc, in_maps, list(range(NCORES)), trace=trace)
    u = inputs["u"]
    B, T, _ = u.shape
    out = np.empty((B, T, C), np.float32)
    for core in range(NCORES):
        b, h = core // 2, core % 2
        out[b, h * TL:(h + 1) * TL, :] = r.results[core]["z_out"].T
    return out, r


def kernel(**inputs):
    out, _ = run_device(inputs)
    return out.astype(np.float32)


# revision 3
# speedup vs baseline: 1.0290x; 1.0290x over previous
"""DEQ transformer block with Anderson acceleration on 8 Trainium2 NeuronCores.

v2: fp16 matmul pipeline.  Each of the 4 sequences (B=4) is split across a
pair of cores (512 tokens each).  K/V halves are exchanged within each pair
via AllGather (fp16) every DEQ iteration.  Attention/in-proj weights are
SBUF-resident fp16 (LayerNorm gamma/beta folded in host-side); MLP weights
stream per iteration.  Softmax denominators ride along the AV matmul via a
ones column appended to V.  All accumulation in fp32 PSUM; LN stats,
softmax normalization and the Anderson solve run in fp32.
"""

import numpy as np

P = 128
TL = 512          # tokens per core (half a sequence)
C = 768
CCN = 6           # C / 128
NH = 12
DH = 64
HPN = 6           # head pairs
NHID = 3072
HCN = 24          # NHID / 128
KCN = 8           # full-seq key chunks (1024 / 128)
MH = 5            # Anderson history window
LN_EPS = 1e-5
NCORES = 8
GROUPS = [[0, 1], [2, 3], [4, 5], [6, 7]]

_CACHE = {}


def _build(num_iters):
    from contextlib import ExitStack
    import concourse.bass as bass  # noqa
    import concourse.mybir as mybir
    import concourse.tile as tile
    from concourse import bacc
    from concourse.masks import make_identity

    FP = mybir.dt.float32
    FPR = mybir.dt.float32r
    H = mybir.dt.float16
    AF = mybir.ActivationFunctionType
    OP = mybir.AluOpType

    nc = bacc.Bacc()

    # ---------------- DRAM I/O ----------------
    u_d = nc.dram_tensor("u_fm", [C, TL], H, kind="ExternalInput")
    qkw_d = nc.dram_tensor("qkw_pack", [12, P, CCN, P], H, kind="ExternalInput")
    vw_d = nc.dram_tensor("vw_pack", [P, CCN, C], H, kind="ExternalInput")
    wo_d = nc.dram_tensor("wo_pack", [CCN, P, CCN, P], H, kind="ExternalInput")
    w1_d = nc.dram_tensor("w1_pack", [HCN, P, CCN, P], H, kind="ExternalInput")
    w2_d = nc.dram_tensor("w2_pack", [CCN, P, HCN, P], H, kind="ExternalInput")
    vb_d = nc.dram_tensor("vbias_bc", [P, C], FP, kind="ExternalInput")
    bqk_d = nc.dram_tensor("bqk_cols", [P, 12], FP, kind="ExternalInput")
    bo_d = nc.dram_tensor("bo_cols", [P, CCN], FP, kind="ExternalInput")
    b1_d = nc.dram_tensor("b1_cols", [P, HCN], FP, kind="ExternalInput")
    b2_d = nc.dram_tensor("b2_cols", [P, CCN], FP, kind="ExternalInput")
    emb_d = nc.dram_tensor("emb_cols", [P, num_iters * CCN], FP, kind="ExternalInput")
    eye_d = nc.dram_tensor("eye28_c", [P, 14, 14], H, kind="ExternalInput")
    o1h_d = nc.dram_tensor("ones1h_c", [1, P], H, kind="ExternalInput")
    von_d = nc.dram_tensor("vones_c", [P, KCN * NH], H, kind="ExternalInput")
    zo_d = nc.dram_tensor("z_out", [C, TL], FP, kind="ExternalOutput")

    # internal DRAM
    kcc = nc.dram_tensor("k_cc", [C, TL], H)
    vcc = nc.dram_tensor("v_cc", [TL, C], H)
    kall = nc.dram_tensor("k_all", [2, C, TL], H)
    vall = nc.dram_tensor("v_all", [2, TL, C], H)
    fh = nc.dram_tensor("f_hist", [MH, C, TL], FP)

    with tile.TileContext(nc) as tc:
        ctx = ExitStack()
        pool = ctx.enter_context(tc.tile_pool(name="pers", bufs=1))
        itp = ctx.enter_context(tc.tile_pool(name="itp", bufs=1))
        vec = ctx.enter_context(tc.tile_pool(name="vec", bufs=3))
        attp = ctx.enter_context(tc.tile_pool(name="attp", bufs=4))
        w1p = ctx.enter_context(tc.tile_pool(name="w1p", bufs=4))
        w2p = ctx.enter_context(tc.tile_pool(name="w2p", bufs=2))
        wkp = ctx.enter_context(tc.tile_pool(name="wkp", bufs=4))
        fpool = ctx.enter_context(tc.tile_pool(name="fpool", bufs=4))
        prodp = ctx.enter_context(tc.tile_pool(name="prodp", bufs=2))
        rowp = ctx.enter_context(tc.tile_pool(name="rowp", bufs=2))
        vrow = ctx.enter_context(tc.tile_pool(name="vrow", bufs=4))
        pA = ctx.enter_context(tc.tile_pool(name="pA", bufs=2, space="PSUM"))
        pB = ctx.enter_context(tc.tile_pool(name="pB", bufs=2, space="PSUM"))
        pC = ctx.enter_context(tc.tile_pool(name="pC", bufs=2, space="PSUM"))

        # ------------- persistent tiles -------------
        vw_sb = pool.tile([P, CCN, C], H, name="vw_sb")
        z_sb = pool.tile([P, CCN, TL], FP, name="z_sb")
        k_sb = pool.tile([P, CCN, 2 * TL], H, name="k_sb")
        v_sb = pool.tile([P, KCN, NH, DH + 1], H, name="v_sb")
        qo_sb = pool.tile([P, CCN, TL], H, name="qo_sb")   # Q, then attn out
        xn_sb = pool.tile([P, CCN, TL], H, name="xn_sb")   # x1, then x2
        attnres = pool.tile([P, CCN, TL], FP, name="attnres")
        g_sb = pool.tile([P, HCN, TL], H, name="g_sb")
        coefbc = pool.tile([P, MH, TL], H, name="coefbc")
        bqk_sb = pool.tile([P, 12], FP, name="bqk_sb")
        bo_sb = pool.tile([P, CCN], FP, name="bo_sb")
        b1_sb = pool.tile([P, HCN], FP, name="b1_sb")
        b2_sb = pool.tile([P, CCN], FP, name="b2_sb")
        emb_sb = pool.tile([P, num_iters * CCN], FP, name="emb_sb")
        vbbc = pool.tile([P, C], FP, name="vbbc")
        u_sb = pool.tile([P, CCN, TL], H, name="u_sb")
        ident = pool.tile([P, P], FP, name="ident")
        ones1h = pool.tile([1, P], H, name="ones1h")
        ones1r = pool.tile([1, P], FPR, name="ones1r")
        onesPr = pool.tile([P, P], FPR, name="onesPr")
        ones2 = pool.tile([P, 2], FPR, name="ones2")
        ones2h = pool.tile([P, 2], H, name="ones2h")
        eye28 = pool.tile([P, 14, 14], H, name="eye28")
        srows = pool.tile([28, TL], FP, name="srows")
        dots_tm = pool.tile([P, 4, 28], FP, name="dots_tm")
        work = pool.tile([P, 4, 28], FP, name="work")
        coef_tm = pool.tile([P, 4, MH], FP, name="coef_tm")
        crows = pool.tile([MH, TL], FPR, name="crows")

        # loads of persistent weights / consts
        nc.sync.dma_start(vw_sb[:], vw_d[:])
        nc.sync.dma_start(bqk_sb[:], bqk_d[:])
        nc.sync.dma_start(bo_sb[:], bo_d[:])
        nc.sync.dma_start(b1_sb[:], b1_d[:])
        nc.sync.dma_start(b2_sb[:], b2_d[:])
        nc.sync.dma_start(emb_sb[:], emb_d[:])
        nc.sync.dma_start(vbbc[:], vb_d[:])
        for ucc in range(CCN):
            nc.sync.dma_start(u_sb[:, ucc], u_d[ucc * P:(ucc + 1) * P, :])
        make_identity(nc, ident[:])
        nc.sync.dma_start(ones1h[:], o1h_d[:])
        onesf = vec.tile([P, P], FP, name="v")
        nc.vector.memset(onesf[:], 1.0)
        nc.scalar.copy(ones1r[:], onesf[0:1, :])
        nc.scalar.copy(onesPr[:], onesf[:])
        nc.scalar.copy(ones2[:], onesf[:, 0:2])
        with nc.allow_low_precision(reason="fp16 ones"):
            nc.scalar.copy(ones2h[:], onesf[:, 0:2])
        nc.sync.dma_start(eye28[:], eye_d[:])
        # ones column for the AV denominator ride-along
        nc.sync.dma_start(v_sb[:, :, :, DH:DH + 1], von_d[:])

        def ecol(it, cc):
            return emb_sb[:, it * CCN + cc:it * CCN + cc + 1]

        TT = nc.vector.tensor_tensor
        TS = nc.vector.tensor_scalar

        def layernorm(src, dst, src_h=False):
            # src [P, CCN, TL] FPR (or fp16); dst [P, CCN, TL] H
            # gamma/beta are folded into the downstream projection weights.
            pmu = pA.tile([2, TL], FP, name="pg")
            pmsq = pA.tile([2, TL], FP, name="pg")
            o2 = ones2h if src_h else ones2
            for cc in range(CCN):
                if src_h:
                    sq = vec.tile([P, TL], H, name="vh")
                    with nc.allow_low_precision(reason="fp16 sq"):
                        nc.scalar.activation(sq[:], src[:, cc], AF.Square)
                else:
                    sq = vec.tile([P, TL], FPR, name="v")
                    nc.scalar.activation(sq[:], src[:, cc].bitcast(FP),
                                         AF.Square)
                nc.tensor.matmul(pmu[:], o2[:], src[:, cc],
                                 start=(cc == 0), stop=(cc == CCN - 1))
                nc.tensor.matmul(pmsq[:], o2[:], sq[:],
                                 start=(cc == 0), stop=(cc == CCN - 1))
            mean_r = vrow.tile([1, TL], FPR, name="vr")
            msq_r = vrow.tile([1, TL], FP, name="vr")
            var_r = vrow.tile([1, TL], FP, name="vr")
            nc.scalar.activation(mean_r[:], pmu[0:1, :], AF.Identity,
                                 scale=1.0 / C)
            nc.scalar.activation(msq_r[:], pmsq[0:1, :], AF.Identity,
                                 scale=1.0 / C)
            TT(out=var_r[:], in0=mean_r[:].bitcast(FP),
               in1=mean_r[:].bitcast(FP), op=OP.mult)
            TT(out=var_r[:], in0=msq_r[:], in1=var_r[:], op=OP.subtract)
            TS(out=var_r[:], in0=var_r[:], scalar1=LN_EPS, scalar2=None,
               op0=OP.add)
            sd_r = vrow.tile([1, TL], FP, name="vr")
            nc.scalar.activation(sd_r[:], var_r[:], AF.Sqrt)
            rstd_f = vrow.tile([1, TL], FP, name="vr")
            nc.vector.reciprocal_approx_fast(rstd_f[:], sd_r[:])
            rstd_r = vrow.tile([1, TL], FPR, name="vr")
            nc.scalar.copy(rstd_r[:], rstd_f[:])
            pmean = pA.tile([P, TL], FP, name="pg")
            prstd = pA.tile([P, TL], FP, name="pg")
            nc.tensor.matmul(pmean[:], ones1r[:], mean_r[:],
                             start=True, stop=True)
            nc.tensor.matmul(prstd[:], ones1r[:], rstd_r[:],
                             start=True, stop=True)
            pmean_sb = vec.tile([P, TL], FP, name="v")
            prstd_sb = vec.tile([P, TL], FP, name="v")
            nc.scalar.copy(pmean_sb[:], pmean[:])
            nc.scalar.copy(prstd_sb[:], prstd[:])
            for cc in range(CCN):
                t1 = vec.tile([P, TL], FP, name="v")
                s_in = src[:, cc] if src_h else src[:, cc].bitcast(FP)
                TT(out=t1[:], in0=s_in, in1=pmean_sb[:],
                   op=OP.subtract)
                with nc.allow_low_precision(reason="fp16 ln out"):
                    TT(out=dst[:, cc], in0=t1[:], in1=prstd_sb[:], op=OP.mult)

        hist = []  # slot ids of stored residuals, oldest..newest
        zctx_carry = None

        for it in range(num_iters):
            s_new = it % MH
            prev = hist[-4:]
            Kn = len(prev)

            # ---- A1: z_ctx = z + u + 0.1*emb_it ; x1 = LN1(z_ctx) ----
            # (for it>0 the zctx chunks were built inside the previous
            # iteration's Anderson update loop, interleaved per chunk)
            if it == 0:
                zctx = itp.tile([P, CCN, TL], H, name="zctxh")
                for cc in range(CCN):
                    with nc.allow_low_precision(reason="fp16 zctx"):
                        TS(out=zctx[:, cc], in0=u_sb[:, cc],
                           scalar1=ecol(it, cc), scalar2=None, op0=OP.add)
            else:
                zctx = zctx_carry

            layernorm(zctx, xn_sb, src_h=True)

            # ---- A2: V projection (token-major), stage + AllGather ----
            for tch in range(4):
                pva = pA.tile([P, TL], FP, name="pg")
                pvb = pA.tile([P, TL], FP, name="pg")
                ts = slice(tch * P, (tch + 1) * P)
                for cc in range(CCN):
                    nc.tensor.matmul(pva[:], xn_sb[:, cc, ts], vw_sb[:, cc, 0:512],
                                     start=(cc == 0), stop=(cc == CCN - 1))
                    nc.tensor.matmul(pvb[:, 0:256], xn_sb[:, cc, ts],
                                     vw_sb[:, cc, 512:768],
                                     start=(cc == 0), stop=(cc == CCN - 1))
                with nc.allow_low_precision(reason="fp16 v"):
                    TT(out=v_sb[:, tch, 0:8, 0:DH], in0=pva[:],
                       in1=vbbc[:, 0:512], op=OP.add)
                    TT(out=v_sb[:, tch, 8:12, 0:DH], in0=pvb[:, 0:256],
                       in1=vbbc[:, 512:768], op=OP.add)
                nc.sync.dma_start(vcc[ts, :], v_sb[:, tch, :, 0:DH])

            nc.gpsimd.collective_compute(
                "AllGather", OP.bypass, replica_groups=GROUPS,
                ins=[vcc[:]], outs=[vall[:]])

            # ---- A2': K projection (local tokens), stage + AllGather ----
            for oc in range(CCN):
                wkt = wkp.tile([P, CCN, P], H, name="wkt")
                nc.sync.dma_start(wkt[:], qkw_d[6 + oc])
                pk = pA.tile([P, TL], FP, name="pg")
                for cc in range(CCN):
                    nc.tensor.matmul(pk[:], wkt[:, cc], xn_sb[:, cc],
                                     start=(cc == 0), stop=(cc == CCN - 1))
                with nc.allow_low_precision(reason="fp16 k"):
                    nc.scalar.activation(k_sb[:, oc, 0:TL], pk[:], AF.Identity,
                                         bias=bqk_sb[:, 6 + oc:7 + oc])
                nc.sync.dma_start(kcc[oc * P:(oc + 1) * P, :],
                                  k_sb[:, oc, 0:TL])

            nc.gpsimd.collective_compute(
                "AllGather", OP.bypass, replica_groups=GROUPS,
                ins=[kcc[:]], outs=[kall[:]])

            # ---- A3: Q projection (overlaps the K/V collectives) ----
            for oc in range(CCN):
                wkt = wkp.tile([P, CCN, P], H, name="wkt")
                nc.sync.dma_start(wkt[:], qkw_d[oc])
                pq = pA.tile([P, TL], FP, name="pg")
                for cc in range(CCN):
                    nc.tensor.matmul(pq[:], wkt[:, cc], xn_sb[:, cc],
                                     start=(cc == 0), stop=(cc == CCN - 1))
                with nc.allow_low_precision(reason="fp16 q"):
                    nc.scalar.activation(qo_sb[:, oc], pq[:], AF.Identity,
                                         bias=bqk_sb[:, oc:oc + 1])

            # K/V gathered readback
            for r in range(2):
                for tch in range(4):
                    nc.sync.dma_start(v_sb[:, 4 * r + tch, :, 0:DH],
                                      vall[r, tch * P:(tch + 1) * P, :])
                for cc in range(CCN):
                    nc.sync.dma_start(k_sb[:, cc, r * TL:(r + 1) * TL],
                                      kall[r, cc * P:(cc + 1) * P, :])

            # ---- B: attention (denominator rides the AV matmul) ----
            for hp in range(HPN):
                ha, hb = 2 * hp, 2 * hp + 1
                pava = pC.tile([P, TL], FP, name="pv")
                pavb = pC.tile([P, TL], FP, name="pv")
                for kc in range(KCN):
                    ks = slice(kc * P, (kc + 1) * P)
                    s2 = pB.tile([P, 2, TL], FP, name="sc2")
                    nc.tensor.matmul(s2[:, 0], k_sb[0:64, hp, ks], qo_sb[0:64, hp],
                                     start=True, stop=True)
                    nc.tensor.matmul(s2[:, 1], k_sb[64:128, hp, ks],
                                     qo_sb[64:128, hp], start=True, stop=True)
                    att2 = attp.tile([P, 2, TL], H, name="att")
                    with nc.allow_low_precision(reason="fp16 att"):
                        nc.scalar.activation(att2[:], s2[:], AF.Exp, scale=0.125)
                    nc.tensor.matmul(pava[0:DH + 1, :], v_sb[:, kc, ha, :],
                                     att2[:, 0], start=(kc == 0), stop=(kc == KCN - 1))
                    nc.tensor.matmul(pavb[0:DH + 1, :], v_sb[:, kc, hb, :],
                                     att2[:, 1], start=(kc == 0), stop=(kc == KCN - 1))
                avs = attp.tile([DH + 1, 2, TL], FPR, name="avs", bufs=2)
                nc.scalar.copy(avs[:, 0], pava[0:DH + 1, :])
                nc.scalar.copy(avs[:, 1], pavb[0:DH + 1, :])
                pbca = pA.tile([P, TL], FP, name="pg")
                pbcb = pA.tile([P, TL], FP, name="pg")
                nc.tensor.matmul(pbca[0:64, :], onesPr[DH:DH + 1, 0:64],
                                 avs[DH:DH + 1, 0], start=True, stop=True)
                nc.tensor.matmul(pbcb[0:64, :], onesPr[DH:DH + 1, 0:64],
                                 avs[DH:DH + 1, 1], start=True, stop=True)
                rbc2 = attp.tile([64, 2, TL], FP, name="rbc", bufs=2)
                nc.vector.reciprocal_approx_fast(rbc2[:, 0], pbca[0:64, :])
                nc.vector.reciprocal_approx_fast(rbc2[:, 1], pbcb[0:64, :])
                with nc.allow_low_precision(reason="fp16 attn out"):
                    TT(out=qo_sb[0:64, hp], in0=avs[0:64, 0].bitcast(FP),
                       in1=rbc2[:, 0], op=OP.mult)
                    TT(out=qo_sb[64:128, hp], in0=avs[0:64, 1].bitcast(FP),
                       in1=rbc2[:, 1], op=OP.mult)

            # ---- C: output projection -> attnres (f32) ----
            for oc in range(CCN):
                wkt = wkp.tile([P, CCN, P], H, name="wkt")
                nc.sync.dma_start(wkt[:], wo_d[oc])
                pp = pA.tile([P, TL], FP, name="pg")
                for ci in range(CCN):
                    nc.tensor.matmul(pp[:], wkt[:, ci], qo_sb[:, ci],
                                     start=(ci == 0), stop=(ci == CCN - 1))
                nc.scalar.activation(attnres[:, oc], pp[:], AF.Identity,
                                     bias=bo_sb[:, oc:oc + 1])

            # ---- D: z_attn = z + attnres ; x2 = LN2(z_attn) ----
            za = itp.tile([P, CCN, TL], FPR, name="zctx")
            for cc in range(CCN):
                if it == 0:
                    nc.vector.tensor_copy(za[:, cc], attnres[:, cc])
                else:
                    TT(out=za[:, cc], in0=z_sb[:, cc], in1=attnres[:, cc],
                       op=OP.add)
            layernorm(za, xn_sb)

            # ---- E: MLP; res = attnres + mlp ----
            for hi in range(HCN):
                w1t = w1p.tile([P, CCN, P], H, name="w1t")
                nc.sync.dma_start(w1t[:], w1_d[hi])
                ph = pA.tile([P, TL], FP, name="pg")
                for cc in range(CCN):
                    nc.tensor.matmul(ph[:], w1t[:, cc], xn_sb[:, cc],
                                     start=(cc == 0), stop=(cc == CCN - 1))
                with nc.allow_low_precision(reason="fp16 gelu"):
                    nc.scalar.activation(g_sb[:, hi], ph[:], AF.Gelu,
                                         bias=b1_sb[:, hi:hi + 1])
            for oc in range(CCN):
                po = pC.tile([P, TL], FP, name="pv")
                for h2 in range(2):
                    w2t = w2p.tile([P, 12, P], H, name="w2t")
                    nc.sync.dma_start(w2t[:], w2_d[oc, :, h2 * 12:(h2 + 1) * 12])
                    for hj in range(12):
                        hi = h2 * 12 + hj
                        nc.tensor.matmul(po[:], w2t[:, hj], g_sb[:, hi],
                                         start=(hi == 0), stop=(hi == HCN - 1))
                t2 = vec.tile([P, TL], FP, name="v")
                TS(out=t2[:], in0=po[:], scalar1=b2_sb[:, oc:oc + 1],
                   scalar2=None, op0=OP.add)
                TT(out=attnres[:, oc], in0=attnres[:, oc], in1=t2[:], op=OP.add)

            # store res as newest history entry
            for cc in range(CCN):
                nc.sync.dma_start(fh[s_new, cc * P:(cc + 1) * P, :], attnres[:, cc])

            # ---- F: Anderson update ----
            if it + 1 < num_iters:
                zctx_carry = itp.tile([P, CCN, TL], H, name="zctxh")

            def emit_zctx(cc):
                if it + 1 >= num_iters:
                    return
                t0n = vec.tile([P, TL], FP, name="v")
                nc.scalar.activation(t0n[:], z_sb[:, cc], AF.Identity,
                                     bias=ecol(it + 1, cc))
                with nc.allow_low_precision(reason="fp16 zctx"):
                    TT(out=zctx_carry[:, cc], in0=t0n[:], in1=u_sb[:, cc],
                       op=OP.add)

            if Kn == 0:
                for cc in range(CCN):
                    nc.vector.tensor_copy(z_sb[:, cc], attnres[:, cc])
                    emit_zctx(cc)
            else:
                pairs = [(i, j) for i in range(Kn) for j in range(i, Kn)]
                pairs += [(k, Kn) for k in range(Kn)]
                NP = len(pairs)

                def dfslot(k, cc, write=False):
                    if k == 0:
                        return k_sb[:, cc, 0:TL]
                    if k == 1:
                        return k_sb[:, cc, TL:2 * TL]
                    if k == 2:
                        return qo_sb[:, cc]
                    return xn_sb[:, cc]

                pd = pA.tile([28, TL], FP, name="pg")
                for cc in range(CCN):
                    dfs = []
                    for k in range(Kn):
                        ft = fpool.tile([P, TL], FP, name="ft")
                        nc.sync.dma_start(
                            ft[:], fh[prev[k], cc * P:(cc + 1) * P, :])
                        with nc.allow_low_precision(reason="fp16 dF"):
                            TT(out=dfslot(k, cc, write=True), in0=ft[:],
                               in1=attnres[:, cc], op=OP.subtract)
                        dfs.append(dfslot(k, cc))
                    for idx, (a, b) in enumerate(pairs):
                        prod = prodp.tile([P, TL], H, name="prod")
                        with nc.allow_low_precision(reason="fp16 dots"):
                            if a == b:
                                nc.scalar.activation(prod[:], dfs[a],
                                                     AF.Square)
                            else:
                                ina = dfs[a]
                                inb = (dfs[b] if b < Kn
                                       else attnres[:, cc])
                                TT(out=prod[:], in0=ina, in1=inb, op=OP.mult)
                        nc.tensor.matmul(pd[0:NP, :], eye28[:, idx, 0:NP],
                                         prod[:],
                                         start=(cc == 0 and idx == 0),
                                         stop=(cc == CCN - 1 and idx == NP - 1),
                                         skip_group_check=True)
                nc.scalar.copy(srows[0:NP, :], pd[0:NP, :])

                # transpose dots to token-major
                for tch in range(4):
                    ptr = pA.tile([P, TL], FP, name="pg")
                    nc.tensor.transpose(ptr[:, 0:NP],
                                        srows[0:NP, tch * P:(tch + 1) * P],
                                        ident[0:NP, 0:NP])
                    nc.scalar.copy(dots_tm[:, tch, 0:NP], ptr[:, 0:NP])

                def pidx_of(a, b):
                    return pairs.index((min(a, b), max(a, b)))

                for a in range(Kn):
                    for b in range(Kn):
                        nc.vector.tensor_copy(work[:, :, a * 4 + b],
                                              dots_tm[:, :, pidx_of(a, b)])
                    TS(out=work[:, :, a * 4 + a], in0=work[:, :, a * 4 + a],
                       scalar1=1e-6, scalar2=None, op0=OP.add)
                    nc.vector.tensor_copy(work[:, :, 16 + a],
                                          dots_tm[:, :, pidx_of(a, Kn)])

                def As(a, b):
                    return work[:, :, a * 4 + b]

                def Bs(k):
                    return work[:, :, 16 + k]

                def Al(k):
                    return work[:, :, 20 + k]

                rin = work[:, :, 24]
                tmp = work[:, :, 25]
                fco = work[:, :, 26]
                for i in range(Kn):
                    nc.vector.reciprocal(rin, As(i, i))
                    for j in range(i + 1, Kn):
                        TT(out=fco, in0=As(j, i), in1=rin, op=OP.mult)
                        for m in range(i, Kn):
                            TT(out=tmp, in0=fco, in1=As(i, m), op=OP.mult)
                            TT(out=As(j, m), in0=As(j, m), in1=tmp,
                               op=OP.subtract)
                        TT(out=tmp, in0=fco, in1=Bs(i), op=OP.mult)
                        TT(out=Bs(j), in0=Bs(j), in1=tmp, op=OP.subtract)
                for i in range(Kn - 1, -1, -1):
                    nc.vector.tensor_copy(tmp, Bs(i))
                    for j in range(i + 1, Kn):
                        TT(out=fco, in0=As(i, j), in1=Al(j), op=OP.mult)
                        TT(out=tmp, in0=tmp, in1=fco, op=OP.subtract)
                    nc.vector.reciprocal(rin, As(i, i))
                    TT(out=Al(i), in0=tmp, in1=rin, op=OP.mult)

                # coeffs: ck = -alpha_k  (c0 = 1 + sum(alpha) folds away:
                # z_new = z + res + sum_k ck*dF_k)
                TS(out=coef_tm[:, :, 0:Kn], in0=work[:, :, 20:20 + Kn],
                   scalar1=-1.0, scalar2=None, op0=OP.mult)

                for tch in range(4):
                    ptr = pA.tile([P, TL], FP, name="pg")
                    nc.tensor.transpose(ptr[0:Kn, 0:P],
                                        coef_tm[:, tch, 0:Kn], ident[:])
                    nc.scalar.copy(crows[0:Kn, tch * P:(tch + 1) * P],
                                   ptr[0:Kn, 0:P])

                for k in range(Kn):
                    cst = rowp.tile([1, TL], FPR, name="cst")
                    nc.sync.dma_start(cst[:], crows[k:k + 1, :])
                    pb = pA.tile([P, TL], FP, name="pg")
                    nc.tensor.matmul(pb[:], ones1r[:], cst[:],
                                     start=True, stop=True)
                    with nc.allow_low_precision(reason="fp16 coef"):
                        nc.scalar.copy(coefbc[:, k, :], pb[:])

                # z += res + sum_k ck*dF_k  (dF cached in dead attn tiles)
                for cc in range(CCN):
                    TT(out=z_sb[:, cc], in0=z_sb[:, cc], in1=attnres[:, cc],
                       op=OP.add)
                    t0 = vec.tile([P, TL], H, name="vh")
                    for k in range(Kn):
                        with nc.allow_low_precision(reason="fp16 upd"):
                            TT(out=t0[:], in0=dfslot(k, cc),
                               in1=coefbc[:, k, :], op=OP.mult)
                        TT(out=z_sb[:, cc], in0=z_sb[:, cc], in1=t0[:],
                           op=OP.add)
                    emit_zctx(cc)

            hist.append(s_new)
            if len(hist) > MH:
                hist.pop(0)

        for cc in range(CCN):
            nc.sync.dma_start(zo_d[cc * P:(cc + 1) * P, :], z_sb[:, cc])

        ctx.close()

    nc.finalize()
    return nc


def _host_pack(inputs, num_iters):
    f32 = np.float32
    f16 = np.float16
    ipw = np.ascontiguousarray(inputs["in_proj_w"], f32)
    ipb = np.ascontiguousarray(inputs["in_proj_b"], f32)
    opw = np.ascontiguousarray(inputs["out_proj_w"], f32)
    opb = np.ascontiguousarray(inputs["out_proj_b"], f32)
    w1 = np.ascontiguousarray(inputs["mlp_w1"], f32)
    b1 = np.ascontiguousarray(inputs["mlp_b1"], f32)
    w2 = np.ascontiguousarray(inputs["mlp_w2"], f32)
    b2 = np.ascontiguousarray(inputs["mlp_b2"], f32)
    emb = np.ascontiguousarray(inputs["iter_emb"], f32)
    ln1_w = np.asarray(inputs["ln1_w"], f32)
    ln1_b = np.asarray(inputs["ln1_b"], f32)
    ln2_w = np.asarray(inputs["ln2_w"], f32)
    ln2_b = np.asarray(inputs["ln2_b"], f32)

    # fold LN1 gamma/beta into in_proj, LN2 into mlp_w1
    ipw_e = ipw * ln1_w[None, :]
    ipb_e = ipb + ipw @ ln1_b
    w1_e = w1 * ln2_w[None, :]
    b1_e = b1 + w1 @ ln2_b

    qkw_pack = np.ascontiguousarray(
        ipw_e[:1536].reshape(12, P, CCN, P).transpose(0, 3, 2, 1)).astype(f16)
    vw_pack = np.ascontiguousarray(
        ipw_e[1536:].T.reshape(CCN, P, C).transpose(1, 0, 2)).astype(f16)
    wo_pack = np.ascontiguousarray(
        opw.reshape(CCN, P, CCN, P).transpose(0, 3, 2, 1)).astype(f16)
    w1_pack = np.ascontiguousarray(
        w1_e.reshape(HCN, P, CCN, P).transpose(0, 3, 2, 1)).astype(f16)
    w2_pack = np.ascontiguousarray(
        w2.reshape(CCN, P, HCN, P).transpose(0, 3, 2, 1)).astype(f16)
    vbias_bc = np.ascontiguousarray(
        np.broadcast_to(ipb_e[1536:].reshape(1, C), (P, C))).astype(f32)
    bqk_cols = np.ascontiguousarray(ipb_e[:1536].reshape(12, P).T)
    bo_cols = np.ascontiguousarray(opb.reshape(CCN, P).T)
    b1_cols = np.ascontiguousarray(b1_e.reshape(HCN, P).T)
    b2_cols = np.ascontiguousarray(b2.reshape(CCN, P).T)
    rows = [min(i, emb.shape[0] - 1) for i in range(num_iters)]
    emb_cols = np.ascontiguousarray(
        (0.1 * emb[rows]).reshape(num_iters, CCN, P).transpose(2, 0, 1)
        .reshape(P, num_iters * CCN))
    eye28_c = np.broadcast_to(np.eye(14, dtype=f16), (P, 14, 14)).copy()
    ones1h_c = np.ones((1, P), f16)
    vones_c = np.ones((P, KCN * NH), f16)
    shared = dict(
        qkw_pack=qkw_pack, vw_pack=vw_pack, wo_pack=wo_pack, w1_pack=w1_pack,
        w2_pack=w2_pack, vbias_bc=vbias_bc, bqk_cols=bqk_cols,
        bo_cols=bo_cols, b1_cols=b1_cols, b2_cols=b2_cols,
        emb_cols=emb_cols, eye28_c=eye28_c, ones1h_c=ones1h_c,
        vones_c=vones_c)
    u = np.ascontiguousarray(inputs["u"], f32)
    in_maps = []
    for core in range(NCORES):
        b, h = core // 2, core % 2
        m = dict(shared)
        m["u_fm"] = np.ascontiguousarray(u[b, h * TL:(h + 1) * TL, :].T).astype(f16)
        in_maps.append(m)
    return in_maps


def run_device(inputs, num_iters=None, trace=False):
    from concourse.bass_utils import run_bass_kernel_spmd
    ni = int(inputs.get("num_iters", 6)) if num_iters is None else num_iters
    if ni not in _CACHE:
        _CACHE[ni] = _build(ni)
    nc = _CACHE[ni]
    in_maps = _host_pack(inputs, ni)
    r = run_bass_kernel_spmd(nc, in_maps, list(range(NCORES)), trace=trace)
    u = inputs["u"]
    B, T, _ = u.shape
    out = np.empty((B, T, C), np.float32)
    for core in range(NCORES):
        b, h = core // 2, core % 2
        out[b, h * TL:(h + 1) * TL, :] = r.results[core]["z_out"].T
    return out, r


def kernel(**inputs):
    out, _ = run_device(inputs)
    return out.astype(np.float32)


# revision 4
# speedup vs baseline: 1.0306x; 1.0016x over previous
"""DEQ transformer block with Anderson acceleration on 8 Trainium2 NeuronCores.

v2: fp16 matmul pipeline.  Each of the 4 sequences (B=4) is split across a
pair of cores (512 tokens each).  K/V halves are exchanged within each pair
via AllGather (fp16) every DEQ iteration.  Attention/in-proj weights are
SBUF-resident fp16 (LayerNorm gamma/beta folded in host-side); MLP weights
stream per iteration.  Softmax denominators ride along the AV matmul via a
ones column appended to V.  All accumulation in fp32 PSUM; LN stats,
softmax normalization and the Anderson solve run in fp32.
"""

import numpy as np

P = 128
TL = 512          # tokens per core (half a sequence)
C = 768
CCN = 6           # C / 128
NH = 12
DH = 64
HPN = 6           # head pairs
NHID = 3072
HCN = 24          # NHID / 128
KCN = 8           # full-seq key chunks (1024 / 128)
MH = 5            # Anderson history window
LN_EPS = 1e-5
NCORES = 8
GROUPS = [[0, 1], [2, 3], [4, 5], [6, 7]]

_CACHE = {}


def _build(num_iters):
    from contextlib import ExitStack
    import concourse.bass as bass  # noqa
    import concourse.mybir as mybir
    import concourse.tile as tile
    from concourse import bacc
    from concourse.masks import make_identity

    FP = mybir.dt.float32
    FPR = mybir.dt.float32r
    H = mybir.dt.float16
    AF = mybir.ActivationFunctionType
    OP = mybir.AluOpType

    nc = bacc.Bacc()

    # ---------------- DRAM I/O ----------------
    u_d = nc.dram_tensor("u_fm", [C, TL], H, kind="ExternalInput")
    qkw_d = nc.dram_tensor("qkw_pack", [12, P, CCN, P], H, kind="ExternalInput")
    vw_d = nc.dram_tensor("vw_pack", [P, CCN, C], H, kind="ExternalInput")
    wo_d = nc.dram_tensor("wo_pack", [CCN, P, CCN, P], H, kind="ExternalInput")
    w1_d = nc.dram_tensor("w1_pack", [HCN, P, CCN, P], H, kind="ExternalInput")
    w2_d = nc.dram_tensor("w2_pack", [CCN, P, HCN, P], H, kind="ExternalInput")
    vb_d = nc.dram_tensor("vbias_bc", [P, C], FP, kind="ExternalInput")
    bqk_d = nc.dram_tensor("bqk_cols", [P, 12], FP, kind="ExternalInput")
    bo_d = nc.dram_tensor("bo_cols", [P, CCN], FP, kind="ExternalInput")
    b1_d = nc.dram_tensor("b1_cols", [P, HCN], FP, kind="ExternalInput")
    b2_d = nc.dram_tensor("b2_cols", [P, CCN], FP, kind="ExternalInput")
    emb_d = nc.dram_tensor("emb_cols", [P, num_iters * CCN], FP, kind="ExternalInput")
    eye_d = nc.dram_tensor("eye28_c", [P, 14, 14], H, kind="ExternalInput")
    o1h_d = nc.dram_tensor("ones1h_c", [1, P], H, kind="ExternalInput")
    von_d = nc.dram_tensor("vones_c", [P, KCN * NH], H, kind="ExternalInput")
    zo_d = nc.dram_tensor("z_out", [C, TL], FP, kind="ExternalOutput")

    # internal DRAM
    kcc = nc.dram_tensor("k_cc", [C, TL], H)
    vcc = nc.dram_tensor("v_cc", [TL, C], H)
    kall = nc.dram_tensor("k_all", [2, C, TL], H)
    vall = nc.dram_tensor("v_all", [2, TL, C], H)
    fh = nc.dram_tensor("f_hist", [MH, C, TL], FP)

    with tile.TileContext(nc) as tc:
        ctx = ExitStack()
        pool = ctx.enter_context(tc.tile_pool(name="pers", bufs=1))
        itp = ctx.enter_context(tc.tile_pool(name="itp", bufs=1))
        vec = ctx.enter_context(tc.tile_pool(name="vec", bufs=3))
        attp = ctx.enter_context(tc.tile_pool(name="attp", bufs=4))
        w1p = ctx.enter_context(tc.tile_pool(name="w1p", bufs=4))
        w2p = ctx.enter_context(tc.tile_pool(name="w2p", bufs=2))
        wkp = ctx.enter_context(tc.tile_pool(name="wkp", bufs=4))
        fpool = ctx.enter_context(tc.tile_pool(name="fpool", bufs=4))
        prodp = ctx.enter_context(tc.tile_pool(name="prodp", bufs=2))
        rowp = ctx.enter_context(tc.tile_pool(name="rowp", bufs=2))
        vrow = ctx.enter_context(tc.tile_pool(name="vrow", bufs=4))
        pA = ctx.enter_context(tc.tile_pool(name="pA", bufs=2, space="PSUM"))
        pB = ctx.enter_context(tc.tile_pool(name="pB", bufs=2, space="PSUM"))
        pC = ctx.enter_context(tc.tile_pool(name="pC", bufs=2, space="PSUM"))

        # ------------- persistent tiles -------------
        vw_sb = pool.tile([P, CCN, C], H, name="vw_sb")
        z_sb = pool.tile([P, CCN, TL], FP, name="z_sb")
        k_sb = pool.tile([P, CCN, 2 * TL], H, name="k_sb")
        v_sb = pool.tile([P, KCN, NH, DH + 1], H, name="v_sb")
        qo_sb = pool.tile([P, CCN, TL], H, name="qo_sb")   # Q, then attn out
        xn_sb = pool.tile([P, CCN, TL], H, name="xn_sb")   # x1, then x2
        attnres = pool.tile([P, CCN, TL], FP, name="attnres")
        g_sb = pool.tile([P, HCN, TL], H, name="g_sb")
        coefbc = pool.tile([P, MH, TL], H, name="coefbc")
        bqk_sb = pool.tile([P, 12], FP, name="bqk_sb")
        bo_sb = pool.tile([P, CCN], FP, name="bo_sb")
        b1_sb = pool.tile([P, HCN], FP, name="b1_sb")
        b2_sb = pool.tile([P, CCN], FP, name="b2_sb")
        emb_sb = pool.tile([P, num_iters * CCN], FP, name="emb_sb")
        vbbc = pool.tile([P, C], FP, name="vbbc")
        u_sb = pool.tile([P, CCN, TL], H, name="u_sb")
        ident = pool.tile([P, P], FP, name="ident")
        ones1h = pool.tile([1, P], H, name="ones1h")
        ones1r = pool.tile([1, P], FPR, name="ones1r")
        onesPr = pool.tile([P, P], FPR, name="onesPr")
        ones2 = pool.tile([P, 2], FPR, name="ones2")
        ones2h = pool.tile([P, 2], H, name="ones2h")
        eye28 = pool.tile([P, 14, 14], H, name="eye28")
        srows = pool.tile([28, TL], FP, name="srows")
        dots_tm = pool.tile([P, 4, 28], FP, name="dots_tm")
        work = pool.tile([P, 4, 28], FP, name="work")
        coef_tm = pool.tile([P, 4, MH], FP, name="coef_tm")
        crows = pool.tile([MH, TL], H, name="crows")

        # loads of persistent weights / consts
        nc.sync.dma_start(vw_sb[:], vw_d[:])
        nc.sync.dma_start(bqk_sb[:], bqk_d[:])
        nc.sync.dma_start(bo_sb[:], bo_d[:])
        nc.sync.dma_start(b1_sb[:], b1_d[:])
        nc.sync.dma_start(b2_sb[:], b2_d[:])
        nc.sync.dma_start(emb_sb[:], emb_d[:])
        nc.sync.dma_start(vbbc[:], vb_d[:])
        for ucc in range(CCN):
            nc.sync.dma_start(u_sb[:, ucc], u_d[ucc * P:(ucc + 1) * P, :])
        make_identity(nc, ident[:])
        nc.sync.dma_start(ones1h[:], o1h_d[:])
        onesf = vec.tile([P, P], FP, name="v")
        nc.vector.memset(onesf[:], 1.0)
        nc.scalar.copy(ones1r[:], onesf[0:1, :])
        nc.scalar.copy(onesPr[:], onesf[:])
        nc.scalar.copy(ones2[:], onesf[:, 0:2])
        with nc.allow_low_precision(reason="fp16 ones"):
            nc.scalar.copy(ones2h[:], onesf[:, 0:2])
        nc.sync.dma_start(eye28[:], eye_d[:])
        # ones column for the AV denominator ride-along
        nc.sync.dma_start(v_sb[:, :, :, DH:DH + 1], von_d[:])

        def ecol(it, cc):
            return emb_sb[:, it * CCN + cc:it * CCN + cc + 1]

        TT = nc.vector.tensor_tensor
        TS = nc.vector.tensor_scalar

        def layernorm(src, dst, src_h=False):
            # src [P, CCN, TL] FPR (or fp16); dst [P, CCN, TL] H
            # gamma/beta are folded into the downstream projection weights.
            pmu = pA.tile([2, TL], FP, name="pg")
            pmsq = pA.tile([2, TL], FP, name="pg")
            o2 = ones2h if src_h else ones2
            for cc in range(CCN):
                if src_h:
                    sq = vec.tile([P, TL], H, name="vh")
                    with nc.allow_low_precision(reason="fp16 sq"):
                        nc.scalar.activation(sq[:], src[:, cc], AF.Square)
                else:
                    sq = vec.tile([P, TL], FPR, name="v")
                    nc.scalar.activation(sq[:], src[:, cc].bitcast(FP),
                                         AF.Square)
                nc.tensor.matmul(pmu[:], o2[:], src[:, cc],
                                 start=(cc == 0), stop=(cc == CCN - 1))
                nc.tensor.matmul(pmsq[:], o2[:], sq[:],
                                 start=(cc == 0), stop=(cc == CCN - 1))
            mean_r = vrow.tile([1, TL], FPR, name="vr")
            msq_r = vrow.tile([1, TL], FP, name="vr")
            var_r = vrow.tile([1, TL], FP, name="vr")
            nc.scalar.activation(mean_r[:], pmu[0:1, :], AF.Identity,
                                 scale=1.0 / C)
            nc.scalar.activation(msq_r[:], pmsq[0:1, :], AF.Identity,
                                 scale=1.0 / C)
            TT(out=var_r[:], in0=mean_r[:].bitcast(FP),
               in1=mean_r[:].bitcast(FP), op=OP.mult)
            TT(out=var_r[:], in0=msq_r[:], in1=var_r[:], op=OP.subtract)
            TS(out=var_r[:], in0=var_r[:], scalar1=LN_EPS, scalar2=None,
               op0=OP.add)
            iv_r = vrow.tile([1, TL], FP, name="vr")
            nc.vector.reciprocal_approx_fast(iv_r[:], var_r[:])
            rstd_r = vrow.tile([1, TL], FPR, name="vr")
            nc.scalar.activation(rstd_r[:], iv_r[:], AF.Sqrt)
            pmean = pA.tile([P, TL], FP, name="pg")
            prstd = pA.tile([P, TL], FP, name="pg")
            nc.tensor.matmul(pmean[:], ones1r[:], mean_r[:],
                             start=True, stop=True)
            nc.tensor.matmul(prstd[:], ones1r[:], rstd_r[:],
                             start=True, stop=True)
            pmean_sb = vec.tile([P, TL], FP, name="pmsb", bufs=1)
            prstd_sb = vec.tile([P, TL], FP, name="prsb", bufs=1)
            nc.scalar.copy(pmean_sb[:], pmean[:])
            nc.scalar.copy(prstd_sb[:], prstd[:])
            for cc in range(CCN):
                t1 = vec.tile([P, TL], FP, name="v")
                s_in = src[:, cc] if src_h else src[:, cc].bitcast(FP)
                TT(out=t1[:], in0=s_in, in1=pmean_sb[:],
                   op=OP.subtract)
                with nc.allow_low_precision(reason="fp16 ln out"):
                    TT(out=dst[:, cc], in0=t1[:], in1=prstd_sb[:], op=OP.mult)

        hist = []  # slot ids of stored residuals, oldest..newest
        zctx_carry = None

        for it in range(num_iters):
            s_new = it % MH
            prev = hist[-4:]
            Kn = len(prev)

            # ---- A1: z_ctx = z + u + 0.1*emb_it ; x1 = LN1(z_ctx) ----
            # (for it>0 the zctx chunks were built inside the previous
            # iteration's Anderson update loop, interleaved per chunk)
            if it == 0:
                zctx = itp.tile([P, CCN, TL], H, name="zctxh")
                for cc in range(CCN):
                    with nc.allow_low_precision(reason="fp16 zctx"):
                        TS(out=zctx[:, cc], in0=u_sb[:, cc],
                           scalar1=ecol(it, cc), scalar2=None, op0=OP.add)
            else:
                zctx = zctx_carry

            layernorm(zctx, xn_sb, src_h=True)

            # ---- A2: V projection (token-major), stage + AllGather ----
            for tch in range(4):
                pva = pA.tile([P, TL], FP, name="pg")
                pvb = pA.tile([P, TL], FP, name="pg")
                ts = slice(tch * P, (tch + 1) * P)
                for cc in range(CCN):
                    nc.tensor.matmul(pva[:], xn_sb[:, cc, ts], vw_sb[:, cc, 0:512],
                                     start=(cc == 0), stop=(cc == CCN - 1))
                    nc.tensor.matmul(pvb[:, 0:256], xn_sb[:, cc, ts],
                                     vw_sb[:, cc, 512:768],
                                     start=(cc == 0), stop=(cc == CCN - 1))
                with nc.allow_low_precision(reason="fp16 v"):
                    TT(out=v_sb[:, tch, 0:8, 0:DH], in0=pva[:],
                       in1=vbbc[:, 0:512], op=OP.add)
                    TT(out=v_sb[:, tch, 8:12, 0:DH], in0=pvb[:, 0:256],
                       in1=vbbc[:, 512:768], op=OP.add)
                nc.sync.dma_start(vcc[ts, :], v_sb[:, tch, :, 0:DH])

            nc.gpsimd.collective_compute(
                "AllGather", OP.bypass, replica_groups=GROUPS,
                ins=[vcc[:]], outs=[vall[:]])

            # ---- A2': K projection (local tokens), stage + AllGather ----
            for oc in range(CCN):
                wkt = wkp.tile([P, CCN, P], H, name="wkt")
                nc.sync.dma_start(wkt[:], qkw_d[6 + oc])
                pk = pA.tile([P, TL], FP, name="pg")
                for cc in range(CCN):
                    nc.tensor.matmul(pk[:], wkt[:, cc], xn_sb[:, cc],
                                     start=(cc == 0), stop=(cc == CCN - 1))
                with nc.allow_low_precision(reason="fp16 k"):
                    nc.scalar.activation(k_sb[:, oc, 0:TL], pk[:], AF.Identity,
                                         bias=bqk_sb[:, 6 + oc:7 + oc])
                nc.sync.dma_start(kcc[oc * P:(oc + 1) * P, :],
                                  k_sb[:, oc, 0:TL])

            nc.gpsimd.collective_compute(
                "AllGather", OP.bypass, replica_groups=GROUPS,
                ins=[kcc[:]], outs=[kall[:]])

            # ---- A3: Q projection (overlaps the K/V collectives) ----
            for oc in range(CCN):
                wkt = wkp.tile([P, CCN, P], H, name="wkt")
                nc.sync.dma_start(wkt[:], qkw_d[oc])
                pq = pA.tile([P, TL], FP, name="pg")
                for cc in range(CCN):
                    nc.tensor.matmul(pq[:], wkt[:, cc], xn_sb[:, cc],
                                     start=(cc == 0), stop=(cc == CCN - 1))
                with nc.allow_low_precision(reason="fp16 q"):
                    nc.scalar.activation(qo_sb[:, oc], pq[:], AF.Identity,
                                         bias=bqk_sb[:, oc:oc + 1])

            # K/V gathered readback
            for r in range(2):
                for tch in range(4):
                    nc.sync.dma_start(v_sb[:, 4 * r + tch, :, 0:DH],
                                      vall[r, tch * P:(tch + 1) * P, :])
                for cc in range(CCN):
                    nc.sync.dma_start(k_sb[:, cc, r * TL:(r + 1) * TL],
                                      kall[r, cc * P:(cc + 1) * P, :])

            # ---- B: attention (denominator rides the AV matmul) ----
            for hp in range(HPN):
                ha, hb = 2 * hp, 2 * hp + 1
                pava = pC.tile([P, TL], FP, name="pv")
                pavb = pC.tile([P, TL], FP, name="pv")
                for kc in range(KCN):
                    ks = slice(kc * P, (kc + 1) * P)
                    s2 = pB.tile([P, 2, TL], FP, name="sc2")
                    nc.tensor.matmul(s2[:, 0], k_sb[0:64, hp, ks], qo_sb[0:64, hp],
                                     start=True, stop=True)
                    nc.tensor.matmul(s2[:, 1], k_sb[64:128, hp, ks],
                                     qo_sb[64:128, hp], start=True, stop=True)
                    att2 = attp.tile([P, 2, TL], H, name="att")
                    with nc.allow_low_precision(reason="fp16 att"):
                        nc.scalar.activation(att2[:], s2[:], AF.Exp, scale=0.125)
                    nc.tensor.matmul(pava[0:DH + 1, :], v_sb[:, kc, ha, :],
                                     att2[:, 0], start=(kc == 0), stop=(kc == KCN - 1))
                    nc.tensor.matmul(pavb[0:DH + 1, :], v_sb[:, kc, hb, :],
                                     att2[:, 1], start=(kc == 0), stop=(kc == KCN - 1))
                avs = attp.tile([DH + 1, 2, TL], FPR, name="avs", bufs=2)
                nc.scalar.copy(avs[:, 0], pava[0:DH + 1, :])
                nc.scalar.copy(avs[:, 1], pavb[0:DH + 1, :])
                pbca = pA.tile([P, TL], FP, name="pg")
                pbcb = pA.tile([P, TL], FP, name="pg")
                nc.tensor.matmul(pbca[0:64, :], onesPr[DH:DH + 1, 0:64],
                                 avs[DH:DH + 1, 0], start=True, stop=True)
                nc.tensor.matmul(pbcb[0:64, :], onesPr[DH:DH + 1, 0:64],
                                 avs[DH:DH + 1, 1], start=True, stop=True)
                rbc2 = attp.tile([64, 2, TL], FP, name="rbc", bufs=2)
                nc.vector.reciprocal_approx_fast(rbc2[:, 0], pbca[0:64, :])
                nc.vector.reciprocal_approx_fast(rbc2[:, 1], pbcb[0:64, :])
                with nc.allow_low_precision(reason="fp16 attn out"):
                    TT(out=qo_sb[0:64, hp], in0=avs[0:64, 0].bitcast(FP),
                       in1=rbc2[:, 0], op=OP.mult)
                    TT(out=qo_sb[64:128, hp], in0=avs[0:64, 1].bitcast(FP),
                       in1=rbc2[:, 1], op=OP.mult)

            # ---- C: output projection -> attnres (f32) ----
            for oc in range(CCN):
                wkt = wkp.tile([P, CCN, P], H, name="wkt")
                nc.sync.dma_start(wkt[:], wo_d[oc])
                pp = pA.tile([P, TL], FP, name="pg")
                for ci in range(CCN):
                    nc.tensor.matmul(pp[:], wkt[:, ci], qo_sb[:, ci],
                                     start=(ci == 0), stop=(ci == CCN - 1))
                nc.scalar.activation(attnres[:, oc], pp[:], AF.Identity,
                                     bias=bo_sb[:, oc:oc + 1])

            # ---- D: z_attn = z + attnres ; x2 = LN2(z_attn) ----
            za = itp.tile([P, CCN, TL], FPR, name="zctx")
            for cc in range(CCN):
                if it == 0:
                    nc.vector.tensor_copy(za[:, cc], attnres[:, cc])
                else:
                    TT(out=za[:, cc], in0=z_sb[:, cc], in1=attnres[:, cc],
                       op=OP.add)
            layernorm(za, xn_sb)

            # ---- E: MLP; res = attnres + mlp ----
            for hi in range(HCN):
                w1t = w1p.tile([P, CCN, P], H, name="w1t")
                nc.sync.dma_start(w1t[:], w1_d[hi])
                ph = pA.tile([P, TL], FP, name="pg")
                for cc in range(CCN):
                    nc.tensor.matmul(ph[:], w1t[:, cc], xn_sb[:, cc],
                                     start=(cc == 0), stop=(cc == CCN - 1))
                with nc.allow_low_precision(reason="fp16 gelu"):
                    nc.scalar.activation(g_sb[:, hi], ph[:], AF.Gelu,
                                         bias=b1_sb[:, hi:hi + 1])
            for oc in range(CCN):
                po = pC.tile([P, TL], FP, name="pv")
                for h2 in range(2):
                    w2t = w2p.tile([P, 12, P], H, name="w2t")
                    nc.sync.dma_start(w2t[:], w2_d[oc, :, h2 * 12:(h2 + 1) * 12])
                    for hj in range(12):
                        hi = h2 * 12 + hj
                        nc.tensor.matmul(po[:], w2t[:, hj], g_sb[:, hi],
                                         start=(hi == 0), stop=(hi == HCN - 1))
                t2 = vec.tile([P, TL], FP, name="v")
                TS(out=t2[:], in0=po[:], scalar1=b2_sb[:, oc:oc + 1],
                   scalar2=None, op0=OP.add)
                TT(out=attnres[:, oc], in0=attnres[:, oc], in1=t2[:], op=OP.add)

            # store res as newest history entry
            for cc in range(CCN):
                nc.sync.dma_start(fh[s_new, cc * P:(cc + 1) * P, :], attnres[:, cc])

            # ---- F: Anderson update ----
            if it + 1 < num_iters:
                zctx_carry = itp.tile([P, CCN, TL], H, name="zctxh")

            def emit_zctx(cc):
                if it + 1 >= num_iters:
                    return
                t0n = vec.tile([P, TL], FP, name="v")
                nc.scalar.activation(t0n[:], z_sb[:, cc], AF.Identity,
                                     bias=ecol(it + 1, cc))
                with nc.allow_low_precision(reason="fp16 zctx"):
                    TT(out=zctx_carry[:, cc], in0=t0n[:], in1=u_sb[:, cc],
                       op=OP.add)

            if Kn == 0:
                for cc in range(CCN):
                    nc.vector.tensor_copy(z_sb[:, cc], attnres[:, cc])
                    emit_zctx(cc)
            else:
                pairs = [(i, j) for i in range(Kn) for j in range(i, Kn)]
                pairs += [(k, Kn) for k in range(Kn)]
                NP = len(pairs)

                def dfslot(k, cc, write=False):
                    if k == 0:
                        return k_sb[:, cc, 0:TL]
                    if k == 1:
                        return k_sb[:, cc, TL:2 * TL]
                    if k == 2:
                        return qo_sb[:, cc]
                    return v_sb[:, cc, 0:8, 0:DH]

                pd = pA.tile([28, TL], FP, name="pg")
                for cc in range(CCN):
                    dfs = []
                    for k in range(Kn):
                        ft = fpool.tile([P, TL], FP, name="ft")
                        nc.sync.dma_start(
                            ft[:], fh[prev[k], cc * P:(cc + 1) * P, :])
                        with nc.allow_low_precision(reason="fp16 dF"):
                            TT(out=dfslot(k, cc, write=True), in0=ft[:],
                               in1=attnres[:, cc], op=OP.subtract)
                        dfs.append(dfslot(k, cc))
                    for idx, (a, b) in enumerate(pairs):
                        prod = prodp.tile([P, TL], H, name="prod")
                        with nc.allow_low_precision(reason="fp16 dots"):
                            if a == b:
                                nc.scalar.activation(prod[:], dfs[a],
                                                     AF.Square)
                            else:
                                ina = dfs[a]
                                inb = (dfs[b] if b < Kn
                                       else attnres[:, cc])
                                TT(out=prod[:], in0=ina, in1=inb, op=OP.mult)
                        nc.tensor.matmul(pd[0:NP, :], eye28[:, idx, 0:NP],
                                         prod[:],
                                         start=(cc == 0 and idx == 0),
                                         stop=(cc == CCN - 1 and idx == NP - 1),
                                         skip_group_check=True)
                nc.scalar.copy(srows[0:NP, :], pd[0:NP, :])

                # transpose dots to token-major
                for tch in range(4):
                    ptr = pA.tile([P, TL], FP, name="pg")
                    nc.tensor.transpose(ptr[:, 0:NP],
                                        srows[0:NP, tch * P:(tch + 1) * P],
                                        ident[0:NP, 0:NP])
                    nc.scalar.copy(dots_tm[:, tch, 0:NP], ptr[:, 0:NP])

                def pidx_of(a, b):
                    return pairs.index((min(a, b), max(a, b)))

                for a in range(Kn):
                    for b in range(Kn):
                        nc.vector.tensor_copy(work[:, :, a * 4 + b],
                                              dots_tm[:, :, pidx_of(a, b)])
                    TS(out=work[:, :, a * 4 + a], in0=work[:, :, a * 4 + a],
                       scalar1=1e-6, scalar2=None, op0=OP.add)
                    nc.vector.tensor_copy(work[:, :, 16 + a],
                                          dots_tm[:, :, pidx_of(a, Kn)])

                def As(a, b):
                    return work[:, :, a * 4 + b]

                def Bs(k):
                    return work[:, :, 16 + k]

                def Al(k):
                    return work[:, :, 20 + k]

                rin = work[:, :, 24]
                tmp = work[:, :, 25]
                fco = work[:, :, 26]
                for i in range(Kn):
                    nc.vector.reciprocal(rin, As(i, i))
                    for j in range(i + 1, Kn):
                        TT(out=fco, in0=As(j, i), in1=rin, op=OP.mult)
                        for m in range(i, Kn):
                            TT(out=tmp, in0=fco, in1=As(i, m), op=OP.mult)
                            TT(out=As(j, m), in0=As(j, m), in1=tmp,
                               op=OP.subtract)
                        TT(out=tmp, in0=fco, in1=Bs(i), op=OP.mult)
                        TT(out=Bs(j), in0=Bs(j), in1=tmp, op=OP.subtract)
                for i in range(Kn - 1, -1, -1):
                    nc.vector.tensor_copy(tmp, Bs(i))
                    for j in range(i + 1, Kn):
                        TT(out=fco, in0=As(i, j), in1=Al(j), op=OP.mult)
                        TT(out=tmp, in0=tmp, in1=fco, op=OP.subtract)
                    nc.vector.reciprocal(rin, As(i, i))
                    TT(out=Al(i), in0=tmp, in1=rin, op=OP.mult)

                # coeffs: ck = -alpha_k  (c0 = 1 + sum(alpha) folds away:
                # z_new = z + res + sum_k ck*dF_k)
                TS(out=coef_tm[:, :, 0:Kn], in0=work[:, :, 20:20 + Kn],
                   scalar1=-1.0, scalar2=None, op0=OP.mult)

                for tch in range(4):
                    ptr = pA.tile([P, TL], FP, name="pg")
                    nc.tensor.transpose(ptr[0:Kn, 0:P],
                                        coef_tm[:, tch, 0:Kn], ident[:])
                    with nc.allow_low_precision(reason="fp16 coef rows"):
                        nc.scalar.copy(crows[0:Kn, tch * P:(tch + 1) * P],
                                       ptr[0:Kn, 0:P])

                for k in range(Kn):
                    cst = rowp.tile([1, TL], H, name="cst")
                    nc.sync.dma_start(cst[:], crows[k:k + 1, :])
                    pb = pA.tile([P, TL], FP, name="pg")
                    nc.tensor.matmul(pb[:], ones1h[:], cst[:],
                                     start=True, stop=True)
                    with nc.allow_low_precision(reason="fp16 coef"):
                        nc.scalar.copy(coefbc[:, k, :], pb[:])

                # z += res + sum_k ck*dF_k  (dF cached in dead attn tiles)
                for cc in range(CCN):
                    TT(out=z_sb[:, cc], in0=z_sb[:, cc], in1=attnres[:, cc],
                       op=OP.add)
                    t0 = vec.tile([P, TL], H, name="vh")
                    for k in range(Kn):
                        with nc.allow_low_precision(reason="fp16 upd"):
                            TT(out=t0[:], in0=dfslot(k, cc),
                               in1=coefbc[:, k, :], op=OP.mult)
                        TT(out=z_sb[:, cc], in0=z_sb[:, cc], in1=t0[:],
                           op=OP.add)
                    emit_zctx(cc)

            hist.append(s_new)
            if len(hist) > MH:
                hist.pop(0)

        for cc in range(CCN):
            nc.sync.dma_start(zo_d[cc * P:(cc + 1) * P, :], z_sb[:, cc])

        ctx.close()

    nc.finalize()
    return nc


def _host_pack(inputs, num_iters):
    f32 = np.float32
    f16 = np.float16
    ipw = np.ascontiguousarray(inputs["in_proj_w"], f32)
    ipb = np.ascontiguousarray(inputs["in_proj_b"], f32)
    opw = np.ascontiguousarray(inputs["out_proj_w"], f32)
    opb = np.ascontiguousarray(inputs["out_proj_b"], f32)
    w1 = np.ascontiguousarray(inputs["mlp_w1"], f32)
    b1 = np.ascontiguousarray(inputs["mlp_b1"], f32)
    w2 = np.ascontiguousarray(inputs["mlp_w2"], f32)
    b2 = np.ascontiguousarray(inputs["mlp_b2"], f32)
    emb = np.ascontiguousarray(inputs["iter_emb"], f32)
    ln1_w = np.asarray(inputs["ln1_w"], f32)
    ln1_b = np.asarray(inputs["ln1_b"], f32)
    ln2_w = np.asarray(inputs["ln2_w"], f32)
    ln2_b = np.asarray(inputs["ln2_b"], f32)

    # fold LN1 gamma/beta into in_proj, LN2 into mlp_w1
    ipw_e = ipw * ln1_w[None, :]
    ipb_e = ipb + ipw @ ln1_b
    w1_e = w1 * ln2_w[None, :]
    b1_e = b1 + w1 @ ln2_b

    qkw_pack = np.ascontiguousarray(
        ipw_e[:1536].reshape(12, P, CCN, P).transpose(0, 3, 2, 1)).astype(f16)
    vw_pack = np.ascontiguousarray(
        ipw_e[1536:].T.reshape(CCN, P, C).transpose(1, 0, 2)).astype(f16)
    wo_pack = np.ascontiguousarray(
        opw.reshape(CCN, P, CCN, P).transpose(0, 3, 2, 1)).astype(f16)
    w1_pack = np.ascontiguousarray(
        w1_e.reshape(HCN, P, CCN, P).transpose(0, 3, 2, 1)).astype(f16)
    w2_pack = np.ascontiguousarray(
        w2.reshape(CCN, P, HCN, P).transpose(0, 3, 2, 1)).astype(f16)
    vbias_bc = np.ascontiguousarray(
        np.broadcast_to(ipb_e[1536:].reshape(1, C), (P, C))).astype(f32)
    bqk_cols = np.ascontiguousarray(ipb_e[:1536].reshape(12, P).T)
    bo_cols = np.ascontiguousarray(opb.reshape(CCN, P).T)
    b1_cols = np.ascontiguousarray(b1_e.reshape(HCN, P).T)
    b2_cols = np.ascontiguousarray(b2.reshape(CCN, P).T)
    rows = [min(i, emb.shape[0] - 1) for i in range(num_iters)]
    emb_cols = np.ascontiguousarray(
        (0.1 * emb[rows]).reshape(num_iters, CCN, P).transpose(2, 0, 1)
        .reshape(P, num_iters * CCN))
    eye28_c = np.broadcast_to(np.eye(14, dtype=f16), (P, 14, 14)).copy()
    ones1h_c = np.ones((1, P), f16)
    vones_c = np.ones((P, KCN * NH), f16)
    shared = dict(
        qkw_pack=qkw_pack, vw_pack=vw_pack, wo_pack=wo_pack, w1_pack=w1_pack,
        w2_pack=w2_pack, vbias_bc=vbias_bc, bqk_cols=bqk_cols,
        bo_cols=bo_cols, b1_cols=b1_cols, b2_cols=b2_cols,
        emb_cols=emb_cols, eye28_c=eye28_c, ones1h_c=ones1h_c,
        vones_c=vones_c)
    u = np.ascontiguousarray(inputs["u"], f32)
    in_maps = []
    for core in range(NCORES):
        b, h = core // 2, core % 2
        m = dict(shared)
        m["u_fm"] = np.ascontiguousarray(u[b, h * TL:(h + 1) * TL, :].T).astype(f16)
        in_maps.append(m)
    return in_maps


def run_device(inputs, num_iters=None, trace=False):
    from concourse.bass_utils import run_bass_kernel_spmd
    ni = int(inputs.get("num_iters", 6)) if num_iters is None else num_iters
    if ni not in _CACHE:
        _CACHE[ni] = _build(ni)
    nc = _CACHE[ni]
    in_maps = _host_pack(inputs, ni)
    r = run_bass_kernel_spmd(nc, in_maps, list(range(NCORES)), trace=trace)
    u = inputs["u"]
    B, T, _ = u.shape
    out = np.empty((B, T, C), np.float32)
    for core in range(NCORES):
        b, h = core // 2, core % 2
        out[b, h * TL:(h + 1) * TL, :] = r.results[core]["z_out"].T
    return out, r


def kernel(**inputs):
    out, _ = run_device(inputs)
    return out.astype(np.float32)


# revision 5
# speedup vs baseline: 1.0313x; 1.0006x over previous
"""DEQ transformer block with Anderson acceleration on 8 Trainium2 NeuronCores.

v2: fp16 matmul pipeline.  Each of the 4 sequences (B=4) is split across a
pair of cores (512 tokens each).  K/V halves are exchanged within each pair
via AllGather (fp16) every DEQ iteration.  Attention/in-proj weights are
SBUF-resident fp16 (LayerNorm gamma/beta folded in host-side); MLP weights
stream per iteration.  Softmax denominators ride along the AV matmul via a
ones column appended to V.  All accumulation in fp32 PSUM; LN stats,
softmax normalization and the Anderson solve run in fp32.
"""

import numpy as np

P = 128
TL = 512          # tokens per core (half a sequence)
C = 768
CCN = 6           # C / 128
NH = 12
DH = 64
HPN = 6           # head pairs
NHID = 3072
HCN = 24          # NHID / 128
KCN = 8           # full-seq key chunks (1024 / 128)
MH = 5            # Anderson history window
LN_EPS = 1e-5
NCORES = 8
GROUPS = [[0, 1], [2, 3], [4, 5], [6, 7]]

_CACHE = {}


def _build(num_iters):
    from contextlib import ExitStack
    import concourse.bass as bass  # noqa
    import concourse.mybir as mybir
    import concourse.tile as tile
    from concourse import bacc
    from concourse.masks import make_identity

    FP = mybir.dt.float32
    FPR = mybir.dt.float32r
    H = mybir.dt.float16
    AF = mybir.ActivationFunctionType
    OP = mybir.AluOpType

    nc = bacc.Bacc()

    # ---------------- DRAM I/O ----------------
    u_d = nc.dram_tensor("u_fm", [C, TL], H, kind="ExternalInput")
    qkw_d = nc.dram_tensor("qkw_pack", [12, P, CCN, P], H, kind="ExternalInput")
    vw_d = nc.dram_tensor("vw_pack", [P, CCN, C], H, kind="ExternalInput")
    wo_d = nc.dram_tensor("wo_pack", [CCN, P, CCN, P], H, kind="ExternalInput")
    w1_d = nc.dram_tensor("w1_pack", [HCN, P, CCN, P], H, kind="ExternalInput")
    w2_d = nc.dram_tensor("w2_pack", [CCN, P, HCN, P], H, kind="ExternalInput")
    vb_d = nc.dram_tensor("vbias_bc", [P, C], FP, kind="ExternalInput")
    bqk_d = nc.dram_tensor("bqk_cols", [P, 12], FP, kind="ExternalInput")
    bo_d = nc.dram_tensor("bo_cols", [P, CCN], FP, kind="ExternalInput")
    b1_d = nc.dram_tensor("b1_cols", [P, HCN], FP, kind="ExternalInput")
    b2_d = nc.dram_tensor("b2_cols", [P, CCN], FP, kind="ExternalInput")
    emb_d = nc.dram_tensor("emb_cols", [P, num_iters * CCN], FP, kind="ExternalInput")
    eye_d = nc.dram_tensor("eye28_c", [P, 14, 14], H, kind="ExternalInput")
    o1h_d = nc.dram_tensor("ones1h_c", [1, P], H, kind="ExternalInput")
    von_d = nc.dram_tensor("vones_c", [P, KCN * NH], H, kind="ExternalInput")
    zo_d = nc.dram_tensor("z_out", [C, TL], FP, kind="ExternalOutput")

    # internal DRAM
    kcc = nc.dram_tensor("k_cc", [C, TL], H)
    vcc = nc.dram_tensor("v_cc", [TL, C], H)
    kall = nc.dram_tensor("k_all", [2, C, TL], H)
    vall = nc.dram_tensor("v_all", [2, TL, C], H)
    fh = nc.dram_tensor("f_hist", [MH, C, TL], FP)

    with tile.TileContext(nc) as tc:
        ctx = ExitStack()
        pool = ctx.enter_context(tc.tile_pool(name="pers", bufs=1))
        itp = ctx.enter_context(tc.tile_pool(name="itp", bufs=1))
        vec = ctx.enter_context(tc.tile_pool(name="vec", bufs=3))
        attp = ctx.enter_context(tc.tile_pool(name="attp", bufs=4))
        w1p = ctx.enter_context(tc.tile_pool(name="w1p", bufs=4))
        w2p = ctx.enter_context(tc.tile_pool(name="w2p", bufs=2))
        wkp = ctx.enter_context(tc.tile_pool(name="wkp", bufs=4))
        fpool = ctx.enter_context(tc.tile_pool(name="fpool", bufs=4))
        prodp = ctx.enter_context(tc.tile_pool(name="prodp", bufs=2))
        rowp = ctx.enter_context(tc.tile_pool(name="rowp", bufs=2))
        vrow = ctx.enter_context(tc.tile_pool(name="vrow", bufs=4))
        pA = ctx.enter_context(tc.tile_pool(name="pA", bufs=2, space="PSUM"))
        pB = ctx.enter_context(tc.tile_pool(name="pB", bufs=2, space="PSUM"))
        pC = ctx.enter_context(tc.tile_pool(name="pC", bufs=2, space="PSUM"))

        # ------------- persistent tiles -------------
        vw_sb = pool.tile([P, CCN, C], H, name="vw_sb")
        z_sb = pool.tile([P, CCN, TL], FP, name="z_sb")
        k_sb = pool.tile([P, CCN, 2 * TL], H, name="k_sb")
        v_sb = pool.tile([P, KCN, NH, DH + 1], H, name="v_sb")
        qo_sb = pool.tile([P, CCN, TL], H, name="qo_sb")   # Q, then attn out
        xn_sb = pool.tile([P, CCN, TL], H, name="xn_sb")   # x1, then x2
        attnres = pool.tile([P, CCN, TL], FP, name="attnres")
        g_sb = pool.tile([P, HCN, TL], H, name="g_sb")
        coefbc = pool.tile([P, MH, TL], H, name="coefbc")
        bqk_sb = pool.tile([P, 12], FP, name="bqk_sb")
        bo_sb = pool.tile([P, CCN], FP, name="bo_sb")
        b1_sb = pool.tile([P, HCN], FP, name="b1_sb")
        b2_sb = pool.tile([P, CCN], FP, name="b2_sb")
        emb_sb = pool.tile([P, num_iters * CCN], FP, name="emb_sb")
        vbbc = pool.tile([P, C], FP, name="vbbc")
        u_sb = pool.tile([P, CCN, TL], H, name="u_sb")
        ident = pool.tile([P, P], FP, name="ident")
        ones1h = pool.tile([1, P], H, name="ones1h")
        ones1r = pool.tile([1, P], FPR, name="ones1r")
        onesPr = pool.tile([P, P], FPR, name="onesPr")
        ones2 = pool.tile([P, 2], FPR, name="ones2")
        ones2h = pool.tile([P, 2], H, name="ones2h")
        eye28 = pool.tile([P, 14, 14], H, name="eye28")
        srows = pool.tile([28, TL], FP, name="srows")
        dots_tm = pool.tile([P, 4, 28], FP, name="dots_tm")
        work = pool.tile([P, 4, 28], FP, name="work")
        coef_tm = pool.tile([P, 4, MH], FP, name="coef_tm")
        crows = pool.tile([MH, TL], H, name="crows")

        # loads of persistent weights / consts
        nc.sync.dma_start(vw_sb[:], vw_d[:])
        nc.sync.dma_start(bqk_sb[:], bqk_d[:])
        nc.sync.dma_start(bo_sb[:], bo_d[:])
        nc.sync.dma_start(b1_sb[:], b1_d[:])
        nc.sync.dma_start(b2_sb[:], b2_d[:])
        nc.sync.dma_start(emb_sb[:], emb_d[:])
        nc.sync.dma_start(vbbc[:], vb_d[:])
        for ucc in range(CCN):
            nc.sync.dma_start(u_sb[:, ucc], u_d[ucc * P:(ucc + 1) * P, :])
        make_identity(nc, ident[:])
        nc.sync.dma_start(ones1h[:], o1h_d[:])
        onesf = vec.tile([P, P], FP, name="v")
        nc.vector.memset(onesf[:], 1.0)
        nc.scalar.copy(ones1r[:], onesf[0:1, :])
        nc.scalar.copy(onesPr[:], onesf[:])
        nc.scalar.copy(ones2[:], onesf[:, 0:2])
        with nc.allow_low_precision(reason="fp16 ones"):
            nc.scalar.copy(ones2h[:], onesf[:, 0:2])
        nc.sync.dma_start(eye28[:], eye_d[:])
        # ones column for the AV denominator ride-along
        nc.sync.dma_start(v_sb[:, :, :, DH:DH + 1], von_d[:])

        def ecol(it, cc):
            return emb_sb[:, it * CCN + cc:it * CCN + cc + 1]

        TT = nc.vector.tensor_tensor
        TS = nc.vector.tensor_scalar

        def layernorm(src, dst, src_h=False):
            # src [P, CCN, TL] FPR (or fp16); dst [P, CCN, TL] H
            # gamma/beta are folded into the downstream projection weights.
            pmu = pA.tile([2, TL], FP, name="pg")
            pmsq = pA.tile([2, TL], FP, name="pg")
            o2 = ones2h if src_h else ones2
            for cc in range(CCN):
                if src_h:
                    sq = vec.tile([P, TL], H, name="vh")
                    with nc.allow_low_precision(reason="fp16 sq"):
                        nc.scalar.activation(sq[:], src[:, cc], AF.Square)
                else:
                    sq = vec.tile([P, TL], FPR, name="v")
                    nc.scalar.activation(sq[:], src[:, cc].bitcast(FP),
                                         AF.Square)
                nc.tensor.matmul(pmu[:], o2[:], src[:, cc],
                                 start=(cc == 0), stop=(cc == CCN - 1))
                nc.tensor.matmul(pmsq[:], o2[:], sq[:],
                                 start=(cc == 0), stop=(cc == CCN - 1))
            mean_r = vrow.tile([1, TL], FPR, name="vr")
            msq_r = vrow.tile([1, TL], FP, name="vr")
            var_r = vrow.tile([1, TL], FP, name="vr")
            nc.scalar.activation(mean_r[:], pmu[0:1, :], AF.Identity,
                                 scale=1.0 / C)
            nc.scalar.activation(msq_r[:], pmsq[0:1, :], AF.Identity,
                                 scale=1.0 / C)
            TT(out=var_r[:], in0=mean_r[:].bitcast(FP),
               in1=mean_r[:].bitcast(FP), op=OP.mult)
            TT(out=var_r[:], in0=msq_r[:], in1=var_r[:], op=OP.subtract)
            TS(out=var_r[:], in0=var_r[:], scalar1=LN_EPS, scalar2=None,
               op0=OP.add)
            iv_r = vrow.tile([1, TL], FP, name="vr")
            nc.vector.reciprocal_approx_fast(iv_r[:], var_r[:])
            rstd_r = vrow.tile([1, TL], FPR, name="vr")
            nc.scalar.activation(rstd_r[:], iv_r[:], AF.Sqrt)
            pmean = pA.tile([P, TL], FP, name="pg")
            prstd = pA.tile([P, TL], FP, name="pg")
            nc.tensor.matmul(pmean[:], ones1r[:], mean_r[:],
                             start=True, stop=True)
            nc.tensor.matmul(prstd[:], ones1r[:], rstd_r[:],
                             start=True, stop=True)
            pmean_sb = vec.tile([P, TL], FP, name="pmsb", bufs=1)
            prstd_sb = vec.tile([P, TL], FP, name="prsb", bufs=1)
            nc.scalar.copy(pmean_sb[:], pmean[:])
            nc.scalar.copy(prstd_sb[:], prstd[:])
            for cc in range(CCN):
                t1 = vec.tile([P, TL], FP, name="v")
                s_in = src[:, cc] if src_h else src[:, cc].bitcast(FP)
                TT(out=t1[:], in0=s_in, in1=pmean_sb[:],
                   op=OP.subtract)
                with nc.allow_low_precision(reason="fp16 ln out"):
                    TT(out=dst[:, cc], in0=t1[:], in1=prstd_sb[:], op=OP.mult)

        hist = []  # slot ids of stored residuals, oldest..newest
        zctx_carry = None

        for it in range(num_iters):
            s_new = it % MH
            prev = hist[-4:]
            Kn = len(prev)

            # ---- A1: z_ctx = z + u + 0.1*emb_it ; x1 = LN1(z_ctx) ----
            # (for it>0 the zctx chunks were built inside the previous
            # iteration's Anderson update loop, interleaved per chunk)
            if it == 0:
                zctx = itp.tile([P, CCN, TL], H, name="zctxh")
                for cc in range(CCN):
                    with nc.allow_low_precision(reason="fp16 zctx"):
                        TS(out=zctx[:, cc], in0=u_sb[:, cc],
                           scalar1=ecol(it, cc), scalar2=None, op0=OP.add)
            else:
                zctx = zctx_carry

            layernorm(zctx, xn_sb, src_h=True)

            # ---- A2: V projection (token-major), stage + AllGather ----
            for tch in range(4):
                pva = pA.tile([P, TL], FP, name="pg")
                pvb = pA.tile([P, TL], FP, name="pg")
                ts = slice(tch * P, (tch + 1) * P)
                for cc in range(CCN):
                    nc.tensor.matmul(pva[:], xn_sb[:, cc, ts], vw_sb[:, cc, 0:512],
                                     start=(cc == 0), stop=(cc == CCN - 1))
                    nc.tensor.matmul(pvb[:, 0:256], xn_sb[:, cc, ts],
                                     vw_sb[:, cc, 512:768],
                                     start=(cc == 0), stop=(cc == CCN - 1))
                with nc.allow_low_precision(reason="fp16 v"):
                    TT(out=v_sb[:, tch, 0:8, 0:DH], in0=pva[:],
                       in1=vbbc[:, 0:512], op=OP.add)
                    TT(out=v_sb[:, tch, 8:12, 0:DH], in0=pvb[:, 0:256],
                       in1=vbbc[:, 512:768], op=OP.add)
                nc.sync.dma_start(vcc[ts, :], v_sb[:, tch, :, 0:DH])

            nc.gpsimd.collective_compute(
                "AllGather", OP.bypass, replica_groups=GROUPS,
                ins=[vcc[:]], outs=[vall[:]])

            # ---- A2': K projection (local tokens), stage + AllGather ----
            for oc in range(CCN):
                wkt = wkp.tile([P, CCN, P], H, name="wkt")
                nc.sync.dma_start(wkt[:], qkw_d[6 + oc])
                pk = pA.tile([P, TL], FP, name="pg")
                for cc in range(CCN):
                    nc.tensor.matmul(pk[:], wkt[:, cc], xn_sb[:, cc],
                                     start=(cc == 0), stop=(cc == CCN - 1))
                with nc.allow_low_precision(reason="fp16 k"):
                    nc.scalar.activation(k_sb[:, oc, 0:TL], pk[:], AF.Identity,
                                         bias=bqk_sb[:, 6 + oc:7 + oc])
                nc.sync.dma_start(kcc[oc * P:(oc + 1) * P, :],
                                  k_sb[:, oc, 0:TL])

            nc.gpsimd.collective_compute(
                "AllGather", OP.bypass, replica_groups=GROUPS,
                ins=[kcc[:]], outs=[kall[:]])

            # ---- A3: Q projection (overlaps the K/V collectives) ----
            for oc in range(CCN):
                wkt = wkp.tile([P, CCN, P], H, name="wkt")
                nc.sync.dma_start(wkt[:], qkw_d[oc])
                pq = pA.tile([P, TL], FP, name="pg")
                for cc in range(CCN):
                    nc.tensor.matmul(pq[:], wkt[:, cc], xn_sb[:, cc],
                                     start=(cc == 0), stop=(cc == CCN - 1))
                with nc.allow_low_precision(reason="fp16 q"):
                    nc.scalar.activation(qo_sb[:, oc], pq[:], AF.Identity,
                                         bias=bqk_sb[:, oc:oc + 1])

            # K/V gathered readback (K first: scores need it before AV)
            for r in range(2):
                for cc in range(CCN):
                    nc.sync.dma_start(k_sb[:, cc, r * TL:(r + 1) * TL],
                                      kall[r, cc * P:(cc + 1) * P, :])
                for tch in range(4):
                    nc.sync.dma_start(v_sb[:, 4 * r + tch, :, 0:DH],
                                      vall[r, tch * P:(tch + 1) * P, :])

            # WO weight prefetch (overlaps attention)
            wo_tiles = []
            for oc in range(CCN):
                wkt = wkp.tile([P, CCN, P], H, name="wkt")
                nc.sync.dma_start(wkt[:], wo_d[oc])
                wo_tiles.append(wkt)

            # ---- B: attention (denominator rides the AV matmul) ----
            for hp in range(HPN):
                ha, hb = 2 * hp, 2 * hp + 1
                pava = pC.tile([P, TL], FP, name="pv")
                pavb = pC.tile([P, TL], FP, name="pv")
                for kc in range(KCN):
                    ks = slice(kc * P, (kc + 1) * P)
                    s2 = pB.tile([P, 2, TL], FP, name="sc2")
                    nc.tensor.matmul(s2[:, 0], k_sb[0:64, hp, ks], qo_sb[0:64, hp],
                                     start=True, stop=True)
                    nc.tensor.matmul(s2[:, 1], k_sb[64:128, hp, ks],
                                     qo_sb[64:128, hp], start=True, stop=True)
                    att2 = attp.tile([P, 2, TL], H, name="att")
                    with nc.allow_low_precision(reason="fp16 att"):
                        nc.scalar.activation(att2[:], s2[:], AF.Exp, scale=0.125)
                    nc.tensor.matmul(pava[0:DH + 1, :], v_sb[:, kc, ha, :],
                                     att2[:, 0], start=(kc == 0), stop=(kc == KCN - 1))
                    nc.tensor.matmul(pavb[0:DH + 1, :], v_sb[:, kc, hb, :],
                                     att2[:, 1], start=(kc == 0), stop=(kc == KCN - 1))
                avs = attp.tile([DH + 1, 2, TL], FPR, name="avs", bufs=2)
                nc.scalar.copy(avs[:, 0], pava[0:DH + 1, :])
                nc.scalar.copy(avs[:, 1], pavb[0:DH + 1, :])
                pbca = pA.tile([P, TL], FP, name="pg")
                pbcb = pA.tile([P, TL], FP, name="pg")
                nc.tensor.matmul(pbca[0:64, :], onesPr[DH:DH + 1, 0:64],
                                 avs[DH:DH + 1, 0], start=True, stop=True)
                nc.tensor.matmul(pbcb[0:64, :], onesPr[DH:DH + 1, 0:64],
                                 avs[DH:DH + 1, 1], start=True, stop=True)
                rbc2 = attp.tile([64, 2, TL], FP, name="rbc", bufs=2)
                nc.vector.reciprocal_approx_fast(rbc2[:, 0], pbca[0:64, :])
                nc.vector.reciprocal_approx_fast(rbc2[:, 1], pbcb[0:64, :])
                with nc.allow_low_precision(reason="fp16 attn out"):
                    TT(out=qo_sb[0:64, hp], in0=avs[0:64, 0].bitcast(FP),
                       in1=rbc2[:, 0], op=OP.mult)
                    TT(out=qo_sb[64:128, hp], in0=avs[0:64, 1].bitcast(FP),
                       in1=rbc2[:, 1], op=OP.mult)

            # ---- C: output projection -> attnres (f32) ----
            for oc in range(CCN):
                wkt = wo_tiles[oc]
                pp = pA.tile([P, TL], FP, name="pg")
                for ci in range(CCN):
                    nc.tensor.matmul(pp[:], wkt[:, ci], qo_sb[:, ci],
                                     start=(ci == 0), stop=(ci == CCN - 1))
                nc.scalar.activation(attnres[:, oc], pp[:], AF.Identity,
                                     bias=bo_sb[:, oc:oc + 1])

            # ---- D: z_attn = z + attnres ; x2 = LN2(z_attn) ----
            za = itp.tile([P, CCN, TL], FPR, name="zctx")
            for cc in range(CCN):
                if it == 0:
                    nc.vector.tensor_copy(za[:, cc], attnres[:, cc])
                else:
                    TT(out=za[:, cc], in0=z_sb[:, cc], in1=attnres[:, cc],
                       op=OP.add)
            layernorm(za, xn_sb)

            # ---- E: MLP; res = attnres + mlp ----
            for hi in range(HCN):
                w1t = w1p.tile([P, CCN, P], H, name="w1t")
                nc.sync.dma_start(w1t[:], w1_d[hi])
                ph = pA.tile([P, TL], FP, name="pg")
                for cc in range(CCN):
                    nc.tensor.matmul(ph[:], w1t[:, cc], xn_sb[:, cc],
                                     start=(cc == 0), stop=(cc == CCN - 1))
                with nc.allow_low_precision(reason="fp16 gelu"):
                    nc.scalar.activation(g_sb[:, hi], ph[:], AF.Gelu,
                                         bias=b1_sb[:, hi:hi + 1])
            for oc in range(CCN):
                po = pC.tile([P, TL], FP, name="pv")
                for h2 in range(2):
                    w2t = w2p.tile([P, 12, P], H, name="w2t")
                    nc.sync.dma_start(w2t[:], w2_d[oc, :, h2 * 12:(h2 + 1) * 12])
                    for hj in range(12):
                        hi = h2 * 12 + hj
                        nc.tensor.matmul(po[:], w2t[:, hj], g_sb[:, hi],
                                         start=(hi == 0), stop=(hi == HCN - 1))
                t2 = vec.tile([P, TL], FP, name="v")
                TS(out=t2[:], in0=po[:], scalar1=b2_sb[:, oc:oc + 1],
                   scalar2=None, op0=OP.add)
                TT(out=attnres[:, oc], in0=attnres[:, oc], in1=t2[:], op=OP.add)

            # store res as newest history entry
            for cc in range(CCN):
                nc.sync.dma_start(fh[s_new, cc * P:(cc + 1) * P, :], attnres[:, cc])

            # ---- F: Anderson update ----
            if it + 1 < num_iters:
                zctx_carry = itp.tile([P, CCN, TL], H, name="zctxh")

            def emit_zctx(cc):
                if it + 1 >= num_iters:
                    return
                t0n = vec.tile([P, TL], FP, name="v")
                nc.scalar.activation(t0n[:], z_sb[:, cc], AF.Identity,
                                     bias=ecol(it + 1, cc))
                with nc.allow_low_precision(reason="fp16 zctx"):
                    TT(out=zctx_carry[:, cc], in0=t0n[:], in1=u_sb[:, cc],
                       op=OP.add)

            if Kn == 0:
                for cc in range(CCN):
                    nc.vector.tensor_copy(z_sb[:, cc], attnres[:, cc])
                    emit_zctx(cc)
            else:
                pairs = [(i, j) for i in range(Kn) for j in range(i, Kn)]
                pairs += [(k, Kn) for k in range(Kn)]
                NP = len(pairs)

                def dfslot(k, cc, write=False):
                    if k == 0:
                        return k_sb[:, cc, 0:TL]
                    if k == 1:
                        return k_sb[:, cc, TL:2 * TL]
                    if k == 2:
                        return qo_sb[:, cc]
                    return v_sb[:, cc, 0:8, 0:DH]

                pd = pA.tile([28, TL], FP, name="pg")
                for cc in range(CCN):
                    dfs = []
                    for k in range(Kn):
                        ft = fpool.tile([P, TL], FP, name="ft")
                        nc.sync.dma_start(
                            ft[:], fh[prev[k], cc * P:(cc + 1) * P, :])
                        with nc.allow_low_precision(reason="fp16 dF"):
                            TT(out=dfslot(k, cc, write=True), in0=ft[:],
                               in1=attnres[:, cc], op=OP.subtract)
                        dfs.append(dfslot(k, cc))
                    for idx, (a, b) in enumerate(pairs):
                        prod = prodp.tile([P, TL], H, name="prod")
                        with nc.allow_low_precision(reason="fp16 dots"):
                            if a == b:
                                nc.scalar.activation(prod[:], dfs[a],
                                                     AF.Square)
                            else:
                                ina = dfs[a]
                                inb = (dfs[b] if b < Kn
                                       else attnres[:, cc])
                                TT(out=prod[:], in0=ina, in1=inb, op=OP.mult)
                        nc.tensor.matmul(pd[0:NP, :], eye28[:, idx, 0:NP],
                                         prod[:],
                                         start=(cc == 0 and idx == 0),
                                         stop=(cc == CCN - 1 and idx == NP - 1),
                                         skip_group_check=True)
                nc.scalar.copy(srows[0:NP, :], pd[0:NP, :])

                # transpose dots to token-major
                for tch in range(4):
                    ptr = pA.tile([P, TL], FP, name="pg")
                    nc.tensor.transpose(ptr[:, 0:NP],
                                        srows[0:NP, tch * P:(tch + 1) * P],
                                        ident[0:NP, 0:NP])
                    nc.scalar.copy(dots_tm[:, tch, 0:NP], ptr[:, 0:NP])

                def pidx_of(a, b):
                    return pairs.index((min(a, b), max(a, b)))

                for a in range(Kn):
                    for b in range(Kn):
                        nc.vector.tensor_copy(work[:, :, a * 4 + b],
                                              dots_tm[:, :, pidx_of(a, b)])
                    TS(out=work[:, :, a * 4 + a], in0=work[:, :, a * 4 + a],
                       scalar1=1e-6, scalar2=None, op0=OP.add)
                    nc.vector.tensor_copy(work[:, :, 16 + a],
                                          dots_tm[:, :, pidx_of(a, Kn)])

                def As(a, b):
                    return work[:, :, a * 4 + b]

                def Bs(k):
                    return work[:, :, 16 + k]

                def Al(k):
                    return work[:, :, 20 + k]

                rin = work[:, :, 24]
                tmp = work[:, :, 25]
                fco = work[:, :, 26]
                for i in range(Kn):
                    nc.vector.reciprocal(rin, As(i, i))
                    for j in range(i + 1, Kn):
                        TT(out=fco, in0=As(j, i), in1=rin, op=OP.mult)
                        for m in range(i, Kn):
                            TT(out=tmp, in0=fco, in1=As(i, m), op=OP.mult)
                            TT(out=As(j, m), in0=As(j, m), in1=tmp,
                               op=OP.subtract)
                        TT(out=tmp, in0=fco, in1=Bs(i), op=OP.mult)
                        TT(out=Bs(j), in0=Bs(j), in1=tmp, op=OP.subtract)
                for i in range(Kn - 1, -1, -1):
                    nc.vector.tensor_copy(tmp, Bs(i))
                    for j in range(i + 1, Kn):
                        TT(out=fco, in0=As(i, j), in1=Al(j), op=OP.mult)
                        TT(out=tmp, in0=tmp, in1=fco, op=OP.subtract)
                    nc.vector.reciprocal(rin, As(i, i))
                    TT(out=Al(i), in0=tmp, in1=rin, op=OP.mult)

                # coeffs: ck = -alpha_k  (c0 = 1 + sum(alpha) folds away:
                # z_new = z + res + sum_k ck*dF_k)
                TS(out=coef_tm[:, :, 0:Kn], in0=work[:, :, 20:20 + Kn],
                   scalar1=-1.0, scalar2=None, op0=OP.mult)

                for tch in range(4):
                    ptr = pA.tile([P, TL], FP, name="pg")
                    nc.tensor.transpose(ptr[0:Kn, 0:P],
                                        coef_tm[:, tch, 0:Kn], ident[:])
                    with nc.allow_low_precision(reason="fp16 coef rows"):
                        nc.scalar.copy(crows[0:Kn, tch * P:(tch + 1) * P],
                                       ptr[0:Kn, 0:P])

                for k in range(Kn):
                    cst = rowp.tile([1, TL], H, name="cst")
                    nc.sync.dma_start(cst[:], crows[k:k + 1, :])
                    pb = pA.tile([P, TL], FP, name="pg")
                    nc.tensor.matmul(pb[:], ones1h[:], cst[:],
                                     start=True, stop=True)
                    with nc.allow_low_precision(reason="fp16 coef"):
                        nc.scalar.copy(coefbc[:, k, :], pb[:])

                # z += res + sum_k ck*dF_k  (dF cached in dead attn tiles)
                for cc in range(CCN):
                    TT(out=z_sb[:, cc], in0=z_sb[:, cc], in1=attnres[:, cc],
                       op=OP.add)
                    t0 = vec.tile([P, TL], H, name="vh")
                    for k in range(Kn):
                        with nc.allow_low_precision(reason="fp16 upd"):
                            TT(out=t0[:], in0=dfslot(k, cc),
                               in1=coefbc[:, k, :], op=OP.mult)
                        TT(out=z_sb[:, cc], in0=z_sb[:, cc], in1=t0[:],
                           op=OP.add)
                    emit_zctx(cc)

            hist.append(s_new)
            if len(hist) > MH:
                hist.pop(0)

        for cc in range(CCN):
            nc.sync.dma_start(zo_d[cc * P:(cc + 1) * P, :], z_sb[:, cc])

        ctx.close()

    nc.finalize()
    return nc


def _host_pack(inputs, num_iters):
    f32 = np.float32
    f16 = np.float16
    ipw = np.ascontiguousarray(inputs["in_proj_w"], f32)
    ipb = np.ascontiguousarray(inputs["in_proj_b"], f32)
    opw = np.ascontiguousarray(inputs["out_proj_w"], f32)
    opb = np.ascontiguousarray(inputs["out_proj_b"], f32)
    w1 = np.ascontiguousarray(inputs["mlp_w1"], f32)
    b1 = np.ascontiguousarray(inputs["mlp_b1"], f32)
    w2 = np.ascontiguousarray(inputs["mlp_w2"], f32)
    b2 = np.ascontiguousarray(inputs["mlp_b2"], f32)
    emb = np.ascontiguousarray(inputs["iter_emb"], f32)
    ln1_w = np.asarray(inputs["ln1_w"], f32)
    ln1_b = np.asarray(inputs["ln1_b"], f32)
    ln2_w = np.asarray(inputs["ln2_w"], f32)
    ln2_b = np.asarray(inputs["ln2_b"], f32)

    # fold LN1 gamma/beta into in_proj, LN2 into mlp_w1
    ipw_e = ipw * ln1_w[None, :]
    ipb_e = ipb + ipw @ ln1_b
    w1_e = w1 * ln2_w[None, :]
    b1_e = b1 + w1 @ ln2_b

    qkw_pack = np.ascontiguousarray(
        ipw_e[:1536].reshape(12, P, CCN, P).transpose(0, 3, 2, 1)).astype(f16)
    vw_pack = np.ascontiguousarray(
        ipw_e[1536:].T.reshape(CCN, P, C).transpose(1, 0, 2)).astype(f16)
    wo_pack = np.ascontiguousarray(
        opw.reshape(CCN, P, CCN, P).transpose(0, 3, 2, 1)).astype(f16)
    w1_pack = np.ascontiguousarray(
        w1_e.reshape(HCN, P, CCN, P).transpose(0, 3, 2, 1)).astype(f16)
    w2_pack = np.ascontiguousarray(
        w2.reshape(CCN, P, HCN, P).transpose(0, 3, 2, 1)).astype(f16)
    vbias_bc = np.ascontiguousarray(
        np.broadcast_to(ipb_e[1536:].reshape(1, C), (P, C))).astype(f32)
    bqk_cols = np.ascontiguousarray(ipb_e[:1536].reshape(12, P).T)
    bo_cols = np.ascontiguousarray(opb.reshape(CCN, P).T)
    b1_cols = np.ascontiguousarray(b1_e.reshape(HCN, P).T)
    b2_cols = np.ascontiguousarray(b2.reshape(CCN, P).T)
    rows = [min(i, emb.shape[0] - 1) for i in range(num_iters)]
    emb_cols = np.ascontiguousarray(
        (0.1 * emb[rows]).reshape(num_iters, CCN, P).transpose(2, 0, 1)
        .reshape(P, num_iters * CCN))
    eye28_c = np.broadcast_to(np.eye(14, dtype=f16), (P, 14, 14)).copy()
    ones1h_c = np.ones((1, P), f16)
    vones_c = np.ones((P, KCN * NH), f16)
    shared = dict(
        qkw_pack=qkw_pack, vw_pack=vw_pack, wo_pack=wo_pack, w1_pack=w1_pack,
        w2_pack=w2_pack, vbias_bc=vbias_bc, bqk_cols=bqk_cols,
        bo_cols=bo_cols, b1_cols=b1_cols, b2_cols=b2_cols,
        emb_cols=emb_cols, eye28_c=eye28_c, ones1h_c=ones1h_c,
        vones_c=vones_c)
    u = np.ascontiguousarray(inputs["u"], f32)
    in_maps = []
    for core in range(NCORES):
        b, h = core // 2, core % 2
        m = dict(shared)
        m["u_fm"] = np.ascontiguousarray(u[b, h * TL:(h + 1) * TL, :].T).astype(f16)
        in_maps.append(m)
    return in_maps


def run_device(inputs, num_iters=None, trace=False):
    from concourse.bass_utils import run_bass_kernel_spmd
    ni = int(inputs.get("num_iters", 6)) if num_iters is None else num_iters
    if ni not in _CACHE:
        _CACHE[ni] = _build(ni)
    nc = _CACHE[ni]
    in_maps = _host_pack(inputs, ni)
    r = run_bass_kernel_spmd(nc, in_maps, list(range(NCORES)), trace=trace)
    u = inputs["u"]
    B, T, _ = u.shape
    out = np.empty((B, T, C), np.float32)
    for core in range(NCORES):
        b, h = core // 2, core % 2
        out[b, h * TL:(h + 1) * TL, :] = r.results[core]["z_out"].T
    return out, r


def kernel(**inputs):
    out, _ = run_device(inputs)
    return out.astype(np.float32)
